# revision 1
# baseline (speedup 1.0000x reference)
"""Trainium2 Bass kernel for the Deter GRU-MLP block (RSSM deter update).

Sharding: data-parallel over batch B=4096 across 8 NeuronCores (512 rows
each), all parameters replicated; no collectives.

Design:
- Activations live transposed in SBUF (features on partitions, batch on the
  512-wide free axis), so every GEMM consumes weights in natural [K, M]
  layout and the whole per-core batch is one moving pass -- zero on-chip
  transposes, each weight element read exactly once.
- Matmuls run as float32r (full rate at moving-dim 512, ~fp32 precision).
  The GRU gate GEMM runs fully in bf16 (weights cast on host, normalized h1
  written as bf16) since its output passes through sigmoid/tanh.
- RMSNorm reduces over the feature axis (= partitions) with ones-vector
  matmuls on the TensorEngine accumulating into a [1, 512] PSUM slot; the
  per-column 1/rms is replicated across partitions on the idle GPSIMD
  (partition_broadcast), which also runs the final silu multiplies so the
  next layer's matmuls unblock in strict block order.
- Norm gains are folded into weights/biases on the host; silu is decomposed
  as w*sigmoid(w) (CoreSim/ACT-table-friendly).
- The block-diagonal hidden layers let one resident [128, 32, 512] region be
  reused in place for deter -> h0 -> h1-raw (Tile's WAR tracking orders it);
  x and bf16-h1n share another slot; deter is re-streamed for the GRU mix.
- Each layer's norm+next-layer blocks are interleaved so the TensorEngine
  never waits for a full normalize pass.

Measured on 8 axon-tunneled trn2 cores: rel-max error 5.4e-4 vs the fp32
reference; TimelineSim (calibrated TRN2 cost model): ~410 us/core.
"""

import os
import sys
from contextlib import ExitStack

import numpy as np
import ml_dtypes as _ml

for _p in ("/opt/trn_rl_repo", "/opt/pypackages"):
    if os.path.isdir(_p) and _p not in sys.path:
        sys.path.insert(0, _p)

os.environ.setdefault("MYCRO_LOCAL_CACHE", "1")

import concourse.bass as bass  # noqa: E402
import concourse.bacc as bacc  # noqa: E402
import concourse.mybir as mybir  # noqa: E402
import concourse.tile as tile  # noqa: E402

# ---- problem constants (hardcoded; kernel.py must be self-contained) ----
P = 128
B = 4096
NCORES = 8
BC = B // NCORES  # 512 batch columns per core
DETER = 4096
STOCH = 1024
ACT_DIM = 32
DEMB = 16
HIDDEN = 512
BLOCKS = 8
OUT_B = DETER // BLOCKS  # 512
IN_B0 = 4 * HIDDEN + OUT_B  # 2560
EPS = 1e-4

ND = DETER // P    # 32 deter k/n tiles
NX = 4 * HIDDEN // P  # 16 x k tiles

# const-block column layout (single [P, 354] DRAM input)
C_BXT, C_GXT = 0, 16
C_BH0, C_GH0, C_BH1, C_GH1 = 32, 64, 96, 128
C_BG, C_BGM1 = 160, 256
C_ONES, C_EPS = 352, 353
C_NCOL = 354

f32 = mybir.dt.float32
f32r = mybir.dt.float32r

_PROG = None


def _r(ap):
    return ap.bitcast(f32r)


def _build_program():
    """Build the single-core SPMD Bass program (same on all 8 cores)."""
    AF = mybir.ActivationFunctionType
    Alu = mybir.AluOpType
    nc = bacc.Bacc(trn_type="TRN2", target_bir_lowering=False, debug=False)

    def din(name, shape):
        return nc.dram_tensor(name, list(shape), f32, kind="ExternalInput").ap()

    dT = din("dT", (DETER, BC))
    sT = din("sT", (STOCH, BC))
    aT = din("aT", (ACT_DIM, BC))
    eT = din("eT", (DEMB, BC))
    W0 = din("W0", (DETER, HIDDEN))
    W1 = din("W1", (STOCH, HIDDEN))
    W2 = din("W2", (ACT_DIM, HIDDEN))
    W3 = din("W3", (DEMB, HIDDEN))
    Wh0 = din("Wh0", (BLOCKS, IN_B0, OUT_B))
    Wh1 = din("Wh1", (BLOCKS, OUT_B, OUT_B))
    bf16 = mybir.dt.bfloat16
    Wg = nc.dram_tensor("Wg", [BLOCKS, OUT_B, 3 * OUT_B], bf16,
                        kind="ExternalInput").ap()
    cst = din("cst", (P, C_NCOL))
    outT = nc.dram_tensor("outT", [DETER, BC], f32, kind="ExternalOutput").ap()

    with tile.TileContext(nc) as tc, ExitStack() as top:
        consts = top.enter_context(tc.tile_pool(name="consts", bufs=1))
        cst_sb = consts.tile([P, C_NCOL], f32)
        nc.sync.dma_start(out=_r(cst_sb), in_=_r(cst))
        bxt_sb = cst_sb[:, C_BXT:C_BXT + 16]
        gxt_sb = cst_sb[:, C_GXT:C_GXT + 16]
        bh0t_sb = cst_sb[:, C_BH0:C_BH0 + 32]
        gh0t_sb = cst_sb[:, C_GH0:C_GH0 + 32]
        bh1t_sb = cst_sb[:, C_BH1:C_BH1 + 32]
        gh1t_sb = cst_sb[:, C_GH1:C_GH1 + 32]
        bgt_sb = cst_sb[:, C_BG:C_BG + 96]
        bgm1_sb = cst_sb[:, C_BGM1:C_BGM1 + 96]
        ones_sb = cst_sb[:, C_ONES:C_ONES + 1]
        eps_sb = cst_sb[:1, C_EPS:C_EPS + 1]

        psum_acc = top.enter_context(tc.tile_pool(name="pacc", bufs=7, space="PSUM"))
        psum_ss = top.enter_context(tc.tile_pool(name="pss", bufs=1, space="PSUM"))

        # resident main region: deter -> h0 -> h1-raw, in place
        mainp = top.enter_context(tc.tile_pool(name="mainp", bufs=1))
        main_sb = mainp.tile([P, ND, BC], f32)
        # norm scratch pools (used by every rmsnorm, incl. inside gates)
        invp = top.enter_context(tc.tile_pool(name="invp", bufs=1))
        invbp = top.enter_context(tc.tile_pool(name="invbp", bufs=2))
        stmpp = top.enter_context(tc.tile_pool(name="stmpp", bufs=5))

        # x (f32, branch concat) and h1-normalized (bf16, gates input)
        # have disjoint lifetimes and the same byte size -- share one slot
        xh1p = top.enter_context(tc.tile_pool(name="xh1p", bufs=1))

        def norm_silu_unit(unit, invb, name, out=None):
            """out (default unit) <- silu(unit * inv), silu(w)=w*sigmoid(w).

            Gains are pre-folded into the weights/biases on the host.
            Per-tile ops so downstream per-tile matmuls unblock as early
            as possible.  Writes are tagged float32r (rounded) since the
            next layer's fp32r matmuls consume them; a bf16 `out` feeds
            the all-bf16 gates GEMM instead.
            """
            for m in range(4):
                t = unit[:, m, :]
                nc.vector.tensor_mul(_r(t), t, invb)
                s = stmpp.tile([P, BC], f32, tag="stmp",
                               name=f"{name}_{m}")
                nc.scalar.activation(out=s, in_=t, func=AF.Sigmoid)
                # final multiply on GPSIMD: keeps the DVE free and keeps
                # this chain in strict block order so the next phase's
                # first matmuls unblock immediately
                if out is None:
                    nc.gpsimd.tensor_mul(_r(t), t, s)
                else:
                    nc.gpsimd.tensor_mul(out[:, m, :], t, s)

        def finish_norm(ss, D):
            """rstd = 1/sqrt(ss/D + eps), broadcast across partitions."""
            sq = invp.tile([1, BC], f32, tag="sq", name="sq")
            nc.scalar.activation(out=sq, in_=ss, func=AF.Sqrt, bias=eps_sb,
                                 scale=1.0 / D)
            inv = sq
            nc.vector.reciprocal(inv, sq)
            # replicate inv across all 128 partitions on the idle GPSIMD
            invb = invbp.tile([P, BC], f32, tag="invb", name="invb")
            nc.gpsimd.partition_broadcast(invb, inv)
            return invb

        # ------------- phase A (branches) + L0 + L1 -------------
        with ExitStack() as mid:
            wpool = mid.enter_context(tc.tile_pool(name="wpool", bufs=7))
            ysqp = mid.enter_context(tc.tile_pool(name="ysqp", bufs=1))

            with ExitStack() as ph_x:
                x_sb = xh1p.tile([P, NX, BC], f32, tag="xh", name="x_sb")

                with ExitStack() as ph_in:
                    sp = ph_in.enter_context(tc.tile_pool(name="sp", bufs=1))
                    sT_sb = sp.tile([P, STOCH // P, BC], f32)
                    aT_sb = sp.tile([ACT_DIM, BC], f32)
                    eT_sb = sp.tile([DEMB, BC], f32)
                    an_sb = sp.tile([ACT_DIM, BC], f32)

                    # --- prologue DMAs, in the order compute consumes them:
                    # tiny inputs + small branch weights first, then stoch/W1,
                    # then deter/W0 interleaved group by group.
                    w3t = sp.tile([DEMB, HIDDEN], f32, tag="w3t",
                                  name="w3t")
                    nc.sync.dma_start(out=_r(eT_sb), in_=_r(eT))
                    nc.sync.dma_start(out=_r(w3t), in_=_r(W3))
                    w2t = sp.tile([ACT_DIM, HIDDEN], f32, tag="w2t",
                                  name="w2t")
                    nc.sync.dma_start(out=aT_sb, in_=aT)
                    nc.sync.dma_start(out=_r(w2t), in_=_r(W2))
                    w1ts = []
                    for t in range(STOCH // 512):
                        nc.sync.dma_start(
                            out=_r(sT_sb[:, 4 * t:4 * t + 4, :]),
                            in_=_r(sT[512 * t:512 * (t + 1), :].rearrange(
                                "(s p) b -> p s b", p=P)))
                        wt = wpool.tile([P, 4, HIDDEN], f32, tag="wslab",
                                        name=f"w1t_{t}")
                        nc.sync.dma_start(
                            out=_r(wt),
                            in_=_r(W1[512 * t:512 * (t + 1), :]
                                   .rearrange("(s p) m -> p s m", p=P)))
                        w1ts.append(wt)
                    w0ts = []
                    for t in range(DETER // 512):
                        nc.sync.dma_start(
                            out=_r(main_sb[:, 4 * t:4 * t + 4, :]),
                            in_=_r(dT[512 * t:512 * (t + 1), :].rearrange(
                                "(s p) b -> p s b", p=P)))
                        wt = wpool.tile([P, 4, HIDDEN], f32, tag="wslab",
                                        name=f"w0t_{t}")
                        nc.sync.dma_start(
                            out=_r(wt),
                            in_=_r(W0[512 * t:512 * (t + 1), :]
                                   .rearrange("(s p) m -> p s m", p=P)))
                        w0ts.append(wt)

                    # prefetch L0 block-0 weights so L0 can start the
                    # moment the branches finish
                    wh0_pre = []
                    for grp in range(IN_B0 // 512):
                        wt = wpool.tile([P, 4, OUT_B], f32, tag="wslab",
                                        name=f"w_h0_0_{grp}")
                        nc.sync.dma_start(
                            out=_r(wt),
                            in_=_r(Wh0[0, 512 * grp:512 * (grp + 1), :]
                                   .rearrange("(s p) m -> p s m", p=P)))
                        wh0_pre.append(wt)

                    # action preprocess: a / max(|a|, 1)
                    ab_t = stmpp.tile([P, BC], f32, tag="stmp", name="ab_t")
                    ab = ab_t[:ACT_DIM, :]
                    nc.scalar.activation(out=ab, in_=aT_sb, func=AF.Abs)
                    nc.vector.tensor_scalar_max(ab, ab, 1.0)
                    nc.vector.reciprocal(ab, ab)
                    nc.vector.tensor_mul(_r(an_sb), aT_sb, ab)

                    # ---- four input branches: Linear -> RMSNorm -> SiLU ----
                    def branch_big(br, K, wts, rhs_tiles):
                        accs = [psum_acc.tile([P, BC], f32, tag="acc",
                                              name=f"acc_br{br}_{m}")
                                for m in range(4)]
                        nk = K // P
                        for kk in range(nk):
                            grp, s = divmod(kk, 4)
                            rhs = rhs_tiles(kk)
                            for m in range(4):
                                nc.tensor.matmul(
                                    accs[m],
                                    lhsT=_r(wts[grp][:, s, m * P:(m + 1) * P]),
                                    rhs=_r(rhs), start=(kk == 0),
                                    stop=(kk == nk - 1))
                        return accs

                    def branch_small(br, wt, rhs):
                        accs = []
                        for m in range(4):
                            acc = psum_acc.tile([P, BC], f32, tag="acc",
                                                name=f"acc_br{br}_{m}")
                            nc.tensor.matmul(acc,
                                             lhsT=_r(wt[:, m * P:(m + 1) * P]),
                                             rhs=_r(rhs), start=True, stop=True)
                            accs.append(acc)
                        return accs

                    def branch_post(br, accs):
                        # bias add into x region, square, partition-reduce
                        for m in range(4):
                            j = 4 * br + m
                            nc.vector.tensor_scalar_add(
                                _r(x_sb[:, j, :]), accs[m],
                                bxt_sb[:, j:j + 1])
                        ysq = ysqp.tile([P, 4, BC], f32, tag="ysq",
                                        name=f"ysq_br{br}")
                        nc.scalar.activation(
                            out=_r(ysq), in_=x_sb[:, 4 * br:4 * br + 4, :],
                            func=AF.Square)
                        ss = psum_ss.tile([1, BC], f32, tag="ss",
                                          name=f"ss_br{br}")
                        for m in range(4):
                            nc.tensor.matmul(ss, lhsT=_r(ones_sb),
                                             rhs=_r(ysq[:, m, :]),
                                             start=(m == 0), stop=(m == 3))
                        invb = finish_norm(ss, HIDDEN)
                        norm_silu_unit(x_sb[:, 4 * br:4 * br + 4, :],
                                       invb, f"st_br{br}")

                    # small branches first (tiny DMAs), then stoch, then deter
                    branch_post(3, branch_small(3, w3t, eT_sb))
                    branch_post(2, branch_small(2, w2t, an_sb))
                    branch_post(1, branch_big(1, STOCH, w1ts,
                                              lambda kk: sT_sb[:, kk, :]))
                    branch_post(0, branch_big(0, DETER, w0ts,
                                              lambda kk: main_sb[:, kk, :]))

                # ---- hidden layer 0: BlockLinear(2560 -> 512/block) ----
                # h0 raw overwrites the deter slices of main_sb in place.
                ss0 = psum_ss.tile([1, BC], f32, tag="ss", name="ss_l0")
                for g in range(BLOCKS):
                    if g == 0:
                        wts = wh0_pre
                    else:
                        wts = []
                        for grp in range(IN_B0 // 512):  # 5 groups
                            wt = wpool.tile([P, 4, OUT_B], f32, tag="wslab",
                                            name=f"w_h0_{g}_{grp}")
                            nc.sync.dma_start(
                                out=_r(wt),
                                in_=_r(Wh0[g, 512 * grp:512 * (grp + 1), :]
                                       .rearrange("(s p) m -> p s m", p=P)))
                            wts.append(wt)
                    accs = [psum_acc.tile([P, BC], f32, tag="acc",
                                          name=f"acc_h0_{g}_{m}")
                            for m in range(4)]
                    nk = IN_B0 // P  # 20
                    for kk in range(nk):
                        grp, s = divmod(kk, 4)
                        rhs = main_sb[:, 4 * g + kk, :] if kk < 4 \
                            else x_sb[:, kk - 4, :]
                        for m in range(4):
                            nc.tensor.matmul(
                                accs[m],
                                lhsT=_r(wts[grp][:, s, m * P:(m + 1) * P]),
                                rhs=_r(rhs), start=(kk == 0),
                                stop=(kk == nk - 1))
                    for m in range(4):
                        j = 4 * g + m
                        nc.vector.tensor_scalar_add(
                            _r(main_sb[:, j, :]), accs[m],
                            bh0t_sb[:, j:j + 1])
                    ysq = ysqp.tile([P, 4, BC], f32, tag="ysq",
                                    name=f"ysq_h0_{g}")
                    nc.scalar.activation(
                        out=_r(ysq), in_=main_sb[:, 4 * g:4 * g + 4, :],
                        func=AF.Square)
                    for m in range(4):
                        nc.tensor.matmul(ss0, lhsT=_r(ones_sb),
                                         rhs=_r(ysq[:, m, :]),
                                         start=(g == 0 and m == 0),
                                         stop=(g == BLOCKS - 1 and m == 3))
                invb0 = finish_norm(ss0, DETER)

                # ---- hidden layer 1, interleaved with the L0 norm so block
                # g's GEMMs start as soon as block g is normalized ----
                ss1 = psum_ss.tile([1, BC], f32, tag="ss", name="ss_l1")
                for g in range(BLOCKS):
                    norm_silu_unit(main_sb[:, 4 * g:4 * g + 4, :],
                                   invb0, f"st_h0_{g}")
                    wt = wpool.tile([P, 4, OUT_B], f32, tag="wslab",
                                    name=f"w_h1_{g}")
                    nc.sync.dma_start(
                        out=_r(wt),
                        in_=_r(Wh1[g].rearrange("(s p) m -> p s m", p=P)))
                    accs = [psum_acc.tile([P, BC], f32, tag="acc",
                                          name=f"acc_h1_{g}_{m}")
                            for m in range(4)]
                    for kk in range(4):
                        rhs = main_sb[:, 4 * g + kk, :]
                        for m in range(4):
                            nc.tensor.matmul(
                                accs[m], lhsT=_r(wt[:, kk, m * P:(m + 1) * P]),
                                rhs=_r(rhs), start=(kk == 0), stop=(kk == 3))
                    for m in range(4):
                        j = 4 * g + m
                        nc.vector.tensor_scalar_add(
                            _r(main_sb[:, j, :]), accs[m],
                            bh1t_sb[:, j:j + 1])
                    ysq = ysqp.tile([P, 4, BC], f32, tag="ysq",
                                    name=f"ysq_h1_{g}")
                    nc.scalar.activation(
                        out=_r(ysq), in_=main_sb[:, 4 * g:4 * g + 4, :],
                        func=AF.Square)
                    for m in range(4):
                        nc.tensor.matmul(ss1, lhsT=_r(ones_sb),
                                         rhs=_r(ysq[:, m, :]),
                                         start=(g == 0 and m == 0),
                                         stop=(g == BLOCKS - 1 and m == 3))
        # ------------- GRU gates + final mix (per block), with the
        # L1 norm interleaved so each block's inputs are ready just in time
        with ExitStack() as ph_g:
            wgp = ph_g.enter_context(tc.tile_pool(name="wgp", bufs=2))
            grup = ph_g.enter_context(tc.tile_pool(name="grup", bufs=2))
            tmpp = ph_g.enter_context(tc.tile_pool(name="tmpp", bufs=2))
            outp = ph_g.enter_context(tc.tile_pool(name="outp", bufs=2))
            drep = ph_g.enter_context(tc.tile_pool(name="drep", bufs=2))

            invb1 = finish_norm(ss1, DETER)
            h1b_sb = xh1p.tile([P, ND, BC], mybir.dt.bfloat16, tag="xh",
                               name="h1b_sb")
            for g in range(BLOCKS):
                norm_silu_unit(main_sb[:, 4 * g:4 * g + 4, :],
                               invb1, f"st_h1_{g}",
                               out=h1b_sb[:, 4 * g:4 * g + 4, :])
                wg = wgp.tile([P, 4, 3 * OUT_B], mybir.dt.bfloat16,
                              tag="wg", name=f"wg_{g}")
                nc.sync.dma_start(
                    out=wg, in_=Wg[g].rearrange("(s p) m -> p s m", p=P))
                dre = drep.tile([P, 4, BC], f32, tag="dre", name=f"dre_{g}")
                nc.sync.dma_start(
                    out=dre,
                    in_=dT[512 * g:512 * (g + 1), :].rearrange(
                        "(s p) b -> p s b", p=P))
                r_sb = grup.tile([P, 4, BC], f32, tag="rc", name=f"r_{g}")
                c_sb = grup.tile([P, 4, BC], f32, tag="rc", name=f"c_{g}")
                u_sb = grup.tile([P, 4, BC], f32, tag="u", name=f"u_{g}")
                for mm in range(12):
                    acc = psum_acc.tile([P, BC], f32, tag="acc",
                                        name=f"acc_g{g}_{mm}")
                    for kk in range(4):
                        nc.tensor.matmul(
                            acc, lhsT=wg[:, kk, mm * P:(mm + 1) * P],
                            rhs=h1b_sb[:, 4 * g + kk, :],
                            start=(kk == 0), stop=(kk == 3))
                    j = 12 * g + mm
                    if mm < 4:
                        nc.scalar.activation(out=r_sb[:, mm, :], in_=acc,
                                             func=AF.Sigmoid,
                                             bias=bgt_sb[:, j:j + 1])
                    elif mm < 8:
                        m = mm - 4
                        nc.vector.scalar_tensor_tensor(
                            out=c_sb[:, m, :], in0=acc,
                            scalar=bgt_sb[:, j:j + 1],
                            in1=r_sb[:, m, :], op0=Alu.add, op1=Alu.mult)
                        nc.scalar.activation(out=c_sb[:, m, :],
                                             in_=c_sb[:, m, :], func=AF.Tanh)
                    else:
                        m = mm - 8
                        nc.scalar.activation(out=u_sb[:, m, :], in_=acc,
                                             func=AF.Sigmoid,
                                             bias=bgm1_sb[:, j:j + 1])
                out_t = outp.tile([P, 4, BC], f32, tag="out", name=f"out_{g}")
                for m in range(4):
                    tmp = tmpp.tile([P, BC], f32, tag="tmp",
                                    name=f"tmp_{g}_{m}")
                    nc.gpsimd.tensor_sub(tmp, c_sb[:, m, :], dre[:, m, :])
                    nc.vector.tensor_mul(tmp, u_sb[:, m, :], tmp)
                    nc.vector.tensor_add(out_t[:, m, :], dre[:, m, :], tmp)
                    # per-tile store: overlaps the remaining mix instead of
                    # waiting for the whole block
                    nc.sync.dma_start(
                        out=outT[512 * g + P * m:512 * g + P * (m + 1), :],
                        in_=out_t[:, m, :])

    nc.compile()
    return nc


def _get_program():
    global _PROG
    if _PROG is None:
        _PROG = _build_program()
    return _PROG


def _make_const_block(inputs):
    f = lambda a: np.asarray(a, dtype=np.float32)
    cst = np.zeros((P, C_NCOL), dtype=np.float32)
    cst[:, C_BXT:C_BXT + 16] = np.stack(
        [f(inputs[b]) * f(inputs[g]) for b, g in
         (("b0", "g0"), ("b1", "g1"), ("b2", "g2"), ("b3", "g3"))]
    ).reshape(16, P).T
    cst[:, C_BH0:C_BH0 + 32] = (
        f(inputs["bh0"]) * f(inputs["gh0"])).reshape(32, P).T
    cst[:, C_BH1:C_BH1 + 32] = (
        f(inputs["bh1"]) * f(inputs["gh1"])).reshape(32, P).T
    bgt = f(inputs["bg"]).reshape(96, P).T
    cst[:, C_BG:C_BG + 96] = bgt
    cst[:, C_BGM1:C_BGM1 + 96] = bgt - 1.0
    cst[:, C_ONES] = 1.0
    cst[:, C_EPS] = EPS
    return cst


def _prep_inputs(inputs):
    """Host-side shard + transpose. Returns per-core input maps."""
    f = lambda a: np.ascontiguousarray(np.asarray(a), dtype=np.float32)
    stoch = f(inputs["stoch"]).reshape(B, -1)
    deter = f(inputs["deter"])
    action = f(inputs["action"])
    d_emb = f(inputs["d_emb"])

    g0, g1 = f(inputs["g0"]), f(inputs["g1"])
    g2, g3 = f(inputs["g2"]), f(inputs["g3"])
    gh0, gh1 = f(inputs["gh0"]), f(inputs["gh1"])
    shared = {
        "W0": f(inputs["W0"]) * g0, "W1": f(inputs["W1"]) * g1,
        "W2": f(inputs["W2"]) * g2, "W3": f(inputs["W3"]) * g3,
        "Wh0": f(inputs["Wh0"]) * gh0.reshape(BLOCKS, 1, OUT_B),
        "Wh1": f(inputs["Wh1"]) * gh1.reshape(BLOCKS, 1, OUT_B),
        "Wg": np.asarray(inputs["Wg"]).astype(_ml.bfloat16),
        "cst": _make_const_block(inputs),
    }
    in_maps = []
    for c in range(NCORES):
        sl = slice(c * BC, (c + 1) * BC)
        m = dict(shared)
        m["dT"] = np.ascontiguousarray(deter[sl].T)
        m["sT"] = np.ascontiguousarray(stoch[sl].T)
        m["aT"] = np.ascontiguousarray(action[sl].T)
        m["eT"] = np.ascontiguousarray(d_emb[sl].T)
        in_maps.append(m)
    return in_maps


def _run(inputs, trace=False):
    from concourse import bass_utils
    nc = _get_program()
    in_maps = _prep_inputs(inputs)
    res = bass_utils.run_bass_kernel_spmd(
        nc, in_maps, core_ids=list(range(NCORES)), trace=trace)
    out = np.empty((B, DETER), dtype=np.float32)
    for c in range(NCORES):
        out[c * BC:(c + 1) * BC, :] = res.results[c]["outT"].T
    return out, res.exec_time_ns


def kernel(**inputs):
    out, _ = _run(inputs, trace=False)
    return out


# ---------------------------------------------------------------------------
# benchmarking helper (test-only; the grading path is kernel() above)
# ---------------------------------------------------------------------------

def _bench_generic(nc, in_maps, iters, n_cores=None):
    """Time repeated device executions with device-resident inputs.

    Returns (per-core outputs list, per_iter_ns).  Mirrors
    bass2jax.run_bass_via_pjrt's multi-core path but keeps inputs on device
    and loops without donation.
    """
    import time
    import jax
    import concourse.mybir as mybir
    from jax.sharding import Mesh, NamedSharding, PartitionSpec
    from jax.experimental.shard_map import shard_map
    from concourse import bass2jax

    bass2jax.install_neuronx_cc_hook()
    if n_cores is None:
        n_cores = len(in_maps)

    in_names, out_names, out_avals = [], [], []
    for alloc in nc.m.functions[0].allocations:
        if not isinstance(alloc, mybir.MemoryLocationSet):
            continue
        name = alloc.memorylocations[0].name
        pid_name = (nc.partition_id_tensor.name
                    if nc.partition_id_tensor else None)
        if alloc.kind == "ExternalInput":
            if name != pid_name:
                in_names.append(name)
        elif alloc.kind == "ExternalOutput":
            out_names.append(name)
            out_avals.append(jax.core.ShapedArray(
                tuple(alloc.tensor_shape), mybir.dt.np(alloc.dtype)))
    n_params = len(in_names)

    pid_name = nc.partition_id_tensor.name if nc.partition_id_tensor else None
    bind_names = in_names + out_names + ([pid_name] if pid_name else [])

    def _body(*args):
        operands = list(args)
        if pid_name:
            operands.append(bass2jax.partition_id_tensor())
        outs = bass2jax._bass_exec_p.bind(
            *operands,
            out_avals=tuple(out_avals),
            in_names=tuple(bind_names),
            out_names=tuple(out_names),
            lowering_input_output_aliases=(),
            sim_require_finite=True,
            sim_require_nnan=True,
            nc=nc,
        )
        return tuple(outs)

    devices = jax.devices()[:n_cores]
    mesh = Mesh(np.asarray(devices), ("core",))
    nshard = NamedSharding(mesh, PartitionSpec("core"))
    sharded = jax.jit(
        shard_map(_body, mesh=mesh,
                  in_specs=(PartitionSpec("core"),) * (n_params + len(out_names)),
                  out_specs=(PartitionSpec("core"),) * len(out_names),
                  check_rep=False),
        keep_unused=True)

    concat_in = [
        jax.device_put(
            np.concatenate([np.asarray(in_maps[c][nm]) for c in range(n_cores)],
                           axis=0), nshard)
        for nm in in_names]
    concat_zeros = [
        jax.device_put(
            np.zeros((n_cores * a.shape[0], *a.shape[1:]), a.dtype), nshard)
        for a in out_avals]

    outs = sharded(*concat_in, *concat_zeros)
    jax.block_until_ready(outs)

    # Paired rounds: time 1 synced execute, then BATCH executes with one
    # sync.  The per-round difference is (BATCH-1) device executions with
    # the dispatch/tunnel cost cancelled; the median over rounds kills the
    # tunnel-latency noise.
    BATCH = 6
    diffs = []
    for _ in range(iters):
        t0 = time.perf_counter()
        outs = sharded(*concat_in, *concat_zeros)
        jax.block_until_ready(outs)
        t1 = time.perf_counter()
        for _ in range(BATCH):
            outs = sharded(*concat_in, *concat_zeros)
        jax.block_until_ready(outs)
        t2 = time.perf_counter()
        diffs.append((t2 - t1) - (t1 - t0))
    diffs.sort()
    per_iter_ns = diffs[len(diffs) // 2] / (BATCH - 1) * 1e9
    return outs, per_iter_ns


_TINY = None


def _tiny_program():
    """A near-noop program with the SAME input/output signature as the real
    kernel, so its per-iteration wall time captures the axon dispatch +
    argument marshaling overhead.  The differential against the real kernel
    is the device execution time."""
    global _TINY
    if _TINY is None:
        nc = bacc.Bacc(trn_type="TRN2", target_bir_lowering=False, debug=False)
        shapes = dict(dT=(DETER, BC), sT=(STOCH, BC), aT=(ACT_DIM, BC),
                      eT=(DEMB, BC), W0=(DETER, HIDDEN), W1=(STOCH, HIDDEN),
                      W2=(ACT_DIM, HIDDEN), W3=(DEMB, HIDDEN),
                      Wh0=(BLOCKS, IN_B0, OUT_B), Wh1=(BLOCKS, OUT_B, OUT_B),
                      cst=(P, C_NCOL))
        aps = {k: nc.dram_tensor(k, list(v), f32, kind="ExternalInput").ap()
               for k, v in shapes.items()}
        nc.dram_tensor("Wg", [BLOCKS, OUT_B, 3 * OUT_B], mybir.dt.bfloat16,
                       kind="ExternalInput")
        outT = nc.dram_tensor("outT", [DETER, BC], f32,
                              kind="ExternalOutput").ap()
        with tile.TileContext(nc) as tc:
            with tc.tile_pool(name="t", bufs=2) as pool:
                t = pool.tile([P, 4, BC], f32)
                nc.sync.dma_start(
                    out=t, in_=aps["dT"][:512, :].rearrange(
                        "(s p) b -> p s b", p=P))
                for g in range(BLOCKS):
                    nc.sync.dma_start(
                        out=outT[512 * g:512 * (g + 1), :].rearrange(
                            "(s p) b -> p s b", p=P),
                        in_=t)
        nc.compile()
        _TINY = nc
    return _TINY


def _bench_overhead(inputs, iters=20):
    """Per-iteration overhead of a same-signature near-noop program."""
    nc = _tiny_program()
    in_maps = _prep_inputs(inputs)
    _, t = _bench_generic(nc, in_maps, iters)
    return t


def _bench(inputs, iters=20):
    nc = _get_program()
    in_maps = _prep_inputs(inputs)
    outs, per_iter_ns = _bench_generic(nc, in_maps, iters)
    res = np.asarray(outs[0]).reshape(NCORES, DETER, BC)
    out = np.empty((B, DETER), dtype=np.float32)
    for c in range(NCORES):
        out[c * BC:(c + 1) * BC, :] = res[c].T
    return out, per_iter_ns



# revision 10
# speedup vs baseline: 1.5096x; 1.5096x over previous
"""Trainium2 Bass kernel for the Deter GRU-MLP block (RSSM deter update).

Sharding: data-parallel over batch B=4096 across 8 NeuronCores (512 rows
each), all parameters replicated; no collectives.

Design (v2, mixed fp8/bf16):
- Activations live transposed in SBUF (features on partitions, batch on the
  512-wide free axis); weights consumed in natural [K, M] layout.
- Big GEMMs run as fp8(e4m3) DoubleRow matmuls (0.5 cyc/row, 2 k-tiles per
  instruction): branch W0/W1, the x-part of the block-diagonal L0, and the
  GRU gate GEMM.  The deter slice of L0 and all of L1 stay bf16 for
  accuracy (numpy-simulated end-to-end rel-max err ~1.2e-2 vs 2e-2 budget).
- All weights are scaled by 64 on the host (fp8 normal range) with norm
  gains folded in; pre-norm PSUM results are therefore 64*y.  The rmsnorm
  absorbs the 64: squares are rescaled to true scale on the Pool engine
  ((y64/4096)*y64) and the 64 folds into the reciprocal-rms; the gate
  nonlinearities use the ACT scale operand (sigmoid(acc/64 + b)).
- Pre-norm y64 is stored bf16; squares are written fp8 so the ones-vector
  sum-of-squares matmuls also run DoubleRow.  silu runs as one batched ACT
  op per 4-tile norm unit, casting to fp8 (or bf16 for the L1 input) on
  write.
- deter is resident in SBUF as bf16 (L0 dg slice + final GRU mix operand);
  a second fp8 copy streams through once for the branch-0 GEMM.  Output is
  written bf16 and upcast on the host.
"""

import os
import sys
from contextlib import ExitStack

import numpy as np
import ml_dtypes as _ml

for _p in ("/opt/trn_rl_repo", "/opt/pypackages"):
    if os.path.isdir(_p) and _p not in sys.path:
        sys.path.insert(0, _p)

os.environ.setdefault("MYCRO_LOCAL_CACHE", "1")

import concourse.bass as bass  # noqa: E402
import concourse.bacc as bacc  # noqa: E402
import concourse.mybir as mybir  # noqa: E402
import concourse.tile as tile  # noqa: E402

# ---- problem constants (hardcoded; kernel.py must be self-contained) ----
P = 128
B = 4096
NCORES = 8
BC = B // NCORES  # 512 batch columns per core
DETER = 4096
STOCH = 1024
ACT_DIM = 32
DEMB = 16
HIDDEN = 512
BLOCKS = 8
OUT_B = DETER // BLOCKS  # 512
IN_B0 = 4 * HIDDEN + OUT_B  # 2560
EPS = 1e-4

ND = DETER // P       # 32 deter k/n tiles
NX = 4 * HIDDEN // P  # 16 x k tiles
WS = 64.0             # weight scale folded into rmsnorm / gate scales

# const-block column layout (single [P, C_NCOL] f32 DRAM input)
C_BX64 = 0            # 16: 64*(branch bias * gain)
C_BH064 = 16          # 32: 64*(bh0 * gh0)
C_BH164 = 48          # 32: 64*(bh1 * gh1)
C_BG = 80             # 96: bg (unscaled, sigmoid bias)
C_BG64 = 176          # 96: 64*bg (cand stt bias)
C_BGM1 = 272          # 96: bg - 1 (update sigmoid bias)
C_EPSH = 368          # 1: 4096*EPS
C_NCOL = 369

f32 = mybir.dt.float32
f32r = mybir.dt.float32r
bf16 = mybir.dt.bfloat16
fp8 = mybir.dt.float8e4

# precision flags (see fp8_sim2.py): bf16 for the L0 deter slice and L1
L0DG_BF16 = True
L1_BF16 = True

_PROG = None


def _r(ap):
    return ap.bitcast(f32r)


def _build_program():
    """Build the single-core SPMD Bass program (same on all 8 cores)."""
    AF = mybir.ActivationFunctionType
    Alu = mybir.AluOpType
    DR = mybir.MatmulPerfMode.DoubleRow
    nc = bacc.Bacc(trn_type="TRN2", target_bir_lowering=False, debug=False)

    def din(name, shape, dt=f32):
        return nc.dram_tensor(name, list(shape), dt, kind="ExternalInput").ap()

    dT8 = din("dT8", (DETER, BC), fp8)
    dTb = din("dTb", (DETER, BC), bf16)
    sT8 = din("sT8", (STOCH, BC), fp8)
    aT = din("aT", (ACT_DIM, BC))
    eT = din("eT", (DEMB, BC))
    W0p = din("W0p", (16, P, 2, HIDDEN), fp8)
    W1p = din("W1p", (P, 4, 2, HIDDEN), fp8)
    W2 = din("W2", (ACT_DIM, HIDDEN))
    W3 = din("W3", (DEMB, HIDDEN))
    Wh0x = din("Wh0x", (BLOCKS, P, 8, 2, OUT_B), fp8)
    if L0DG_BF16:
        Wh0d = din("Wh0d", (BLOCKS, P, 4, OUT_B), bf16)
    else:
        Wh0d = din("Wh0d", (BLOCKS, P, 2, 2, OUT_B), fp8)
    if L1_BF16:
        Wh1 = din("Wh1", (BLOCKS, P, 4, OUT_B), bf16)
    else:
        Wh1 = din("Wh1", (BLOCKS, P, 2, 2, OUT_B), fp8)
    Wgp = din("Wgp", (BLOCKS, P, 2, 2, 3 * OUT_B), fp8)
    cst = din("cst", (P, C_NCOL))
    cst8 = din("cst8", (P, 2), fp8)
    outT = nc.dram_tensor("outT", [DETER, BC], bf16,
                          kind="ExternalOutput").ap()

    with tile.TileContext(nc) as tc, ExitStack() as top:
        consts = top.enter_context(tc.tile_pool(name="consts", bufs=1))
        cst_sb = consts.tile([P, C_NCOL], f32)
        nc.sync.dma_start(out=_r(cst_sb), in_=_r(cst))
        ones8 = consts.tile([P, 2], fp8)
        nc.sync.dma_start(out=ones8, in_=cst8)
        bgt = cst_sb[:, C_BG:C_BG + 96]
        bg64 = cst_sb[:, C_BG64:C_BG64 + 96]
        bgm1 = cst_sb[:, C_BGM1:C_BGM1 + 96]
        epsh = cst_sb[:1, C_EPSH:C_EPSH + 1]

        psum_acc = top.enter_context(
            tc.tile_pool(name="pacc", bufs=7, space="PSUM"))
        psum_ss = top.enter_context(
            tc.tile_pool(name="pss", bufs=1, space="PSUM"))

        # resident regions
        mainp = top.enter_context(tc.tile_pool(name="mainp", bufs=1))
        main_sb = mainp.tile([P, ND, BC], bf16)   # pre-norm y64 / h0n
        dtbp = top.enter_context(tc.tile_pool(name="dtbp", bufs=1))
        dtb_sb = dtbp.tile([P, ND, BC], bf16)     # deter bf16 (L0 dg + mix)
        x8p = top.enter_context(tc.tile_pool(name="x8p", bufs=1))
        x8_sb = x8p.tile([P, NX, BC], fp8)        # branch outputs (L0 rhs)
        h1p = top.enter_context(tc.tile_pool(name="h1p", bufs=1))
        h1n8 = h1p.tile([P, ND, BC], fp8)         # L1 normalized (gates rhs)

        ysqp = top.enter_context(tc.tile_pool(name="ysqp", bufs=2))
        invp = top.enter_context(tc.tile_pool(name="invp", bufs=2))
        invbp = top.enter_context(tc.tile_pool(name="invbp", bufs=2))

        def unit_post(unit_y, accs, b64col0, ss, ss_first, ss_last, name):
            """copy accs (+64*bias) into unit_y (bf16); squares at true
            scale (Pool, fp8); DoubleRow ones-reduction into ss."""
            for m in range(4):
                nc.vector.tensor_scalar_add(
                    unit_y[:, m, :], accs[m],
                    cst_sb[:, b64col0 + m:b64col0 + m + 1])
            ysq = ysqp.tile([P, 4, BC], fp8, tag="ysq", name=f"ysq_{name}")
            flat_y = unit_y.rearrange("p a b -> p (a b)")
            nc.gpsimd.scalar_tensor_tensor(
                out=ysq.rearrange("p a b -> p (a b)"), in0=flat_y,
                scalar=1.0 / (WS * WS), in1=flat_y,
                op0=Alu.mult, op1=Alu.mult)
            for half in range(2):
                nc.tensor.matmul(
                    ss, lhsT=ones8, rhs=ysq[:, 2 * half:2 * half + 2, :],
                    start=(ss_first and half == 0),
                    stop=(ss_last and half == 1), perf_mode=DR)

        def finish_norm(ss, D, name):
            """invb = 1/(64*sqrt(ss/D + eps)) broadcast bf16."""
            sq = invp.tile([1, BC], f32, tag="sq", name=f"sq_{name}")
            nc.scalar.activation(out=sq, in_=ss, func=AF.Sqrt, bias=epsh,
                                 scale=(WS * WS) / D)
            inv1 = invp.tile([1, BC], bf16, tag="inv1", name=f"inv1_{name}")
            with nc.allow_low_precision(reason="bf16 rstd broadcast"):
                nc.vector.reciprocal(inv1, sq)
            invb = invbp.tile([P, BC], bf16, tag="invb", name=f"invb_{name}")
            nc.gpsimd.partition_broadcast(invb, inv1)
            return invb

        sigp = top.enter_context(tc.tile_pool(name="sigp", bufs=2))

        def norm_silu_unit(unit_y, invb, out_unit, name):
            """out_unit <- silu(unit_y * invb) = t*sigmoid(t), t=y*inv.
            Per-tile DVE norm-mul (bf16, in place), batched ACT sigmoid,
            batched DVE multiply with cast-on-write."""
            for m in range(4):
                nc.vector.tensor_mul(unit_y[:, m, :], unit_y[:, m, :], invb)
            s = sigp.tile([P, 4, BC], bf16, tag="sig", name=f"sig_{name}")
            nc.scalar.activation(
                out=s.rearrange("p a b -> p (a b)"),
                in_=unit_y.rearrange("p a b -> p (a b)"), func=AF.Sigmoid)
            nc.vector.tensor_mul(
                out_unit.rearrange("p a b -> p (a b)"),
                unit_y.rearrange("p a b -> p (a b)"),
                s.rearrange("p a b -> p (a b)"))

        # ------------- phase A: four input branches -------------
        with ExitStack() as ph_a:
            w0p_pool = ph_a.enter_context(tc.tile_pool(name="w0p", bufs=8))
            d8p = ph_a.enter_context(tc.tile_pool(name="d8p", bufs=8))
            sp = ph_a.enter_context(tc.tile_pool(name="sp", bufs=1))

            sT_sb = sp.tile([P, STOCH // P, BC], fp8)
            aT_sb = sp.tile([ACT_DIM, BC], f32)
            eT_sb = sp.tile([DEMB, BC], f32)
            an_sb = sp.tile([ACT_DIM, BC], f32)
            w3t = sp.tile([DEMB, HIDDEN], f32)
            w2t = sp.tile([ACT_DIM, HIDDEN], f32)
            w1t = sp.tile([P, 4, 2, HIDDEN], fp8)

            # prologue DMAs in consumption order
            nc.sync.dma_start(out=_r(eT_sb), in_=_r(eT))
            nc.sync.dma_start(out=_r(w3t), in_=_r(W3))
            nc.sync.dma_start(out=aT_sb, in_=aT)
            nc.sync.dma_start(out=_r(w2t), in_=_r(W2))
            nc.sync.dma_start(out=sT_sb,
                              in_=sT8.rearrange("(s p) b -> p s b", p=P))
            nc.sync.dma_start(out=w1t, in_=W1p)
            d8_chunks = []
            w0_slabs = []
            for c in range(8):
                d8 = d8p.tile([P, 4, BC], fp8, tag="d8", name=f"d8_{c}")
                nc.sync.dma_start(
                    out=d8, in_=dT8[512 * c:512 * (c + 1), :].rearrange(
                        "(s p) b -> p s b", p=P))
                d8_chunks.append(d8)
                w0 = w0p_pool.tile([P, 2, 2, HIDDEN], fp8, tag="w0",
                                   name=f"w0_{c}")
                nc.sync.dma_start(
                    out=w0,
                    in_=W0p[2 * c:2 * c + 2].rearrange("s p j m -> p s j m"))
                w0_slabs.append(w0)
            # action preprocess: a / max(|a|, 1)
            ab = sp.tile([ACT_DIM, BC], f32)
            nc.scalar.activation(out=ab, in_=aT_sb, func=AF.Abs)
            nc.vector.tensor_scalar_max(ab, ab, 1.0)
            nc.vector.reciprocal(ab, ab)
            nc.vector.tensor_mul(_r(an_sb), aT_sb, ab)

            def branch_small(br, wt, rhs):
                accs = []
                for m in range(4):
                    acc = psum_acc.tile([P, BC], f32, tag="acc",
                                        name=f"acc_br{br}_{m}")
                    nc.tensor.matmul(acc,
                                     lhsT=_r(wt[:, m * P:(m + 1) * P]),
                                     rhs=_r(rhs), start=True, stop=True)
                    accs.append(acc)
                return accs

            def branch_dr(br, npairs, wslab, rhs_pair):
                accs = [psum_acc.tile([P, BC], f32, tag="acc",
                                      name=f"acc_br{br}_{m}")
                        for m in range(4)]
                for kp in range(npairs):
                    w = wslab(kp)
                    rhs = rhs_pair(kp)
                    for m in range(4):
                        nc.tensor.matmul(
                            accs[m], lhsT=w[:, :, m * P:(m + 1) * P],
                            rhs=rhs, start=(kp == 0),
                            stop=(kp == npairs - 1), perf_mode=DR)
                return accs

            def branch_post(br, accs):
                ssb = psum_ss.tile([1, BC], f32, tag="ss", name=f"ss_br{br}")
                unit = main_sb[:, 4 * br:4 * br + 4, :]
                unit_post(unit, accs, C_BX64 + 4 * br, ssb, True, True,
                          f"br{br}")
                invb = finish_norm(ssb, HIDDEN, f"br{br}")
                norm_silu_unit(unit, invb, x8_sb[:, 4 * br:4 * br + 4, :],
                               f"br{br}")

            branch_post(3, branch_small(3, w3t, eT_sb))
            branch_post(2, branch_small(2, w2t, an_sb))
            branch_post(1, branch_dr(
                1, 4, lambda kp: w1t[:, kp, :, :],
                lambda kp: sT_sb[:, 2 * kp:2 * kp + 2, :]))
            branch_post(0, branch_dr(
                0, 16, lambda kp: w0_slabs[kp // 2][:, kp % 2, :, :],
                lambda kp: d8_chunks[kp // 2][
                    :, 2 * (kp % 2):2 * (kp % 2) + 2, :]))

        # ------------- L0: BlockLinear(2560 -> 512/block) -------------
        with ExitStack() as ph_l:
            wh0xp = ph_l.enter_context(tc.tile_pool(name="wh0xp", bufs=2))
            wh0dp = ph_l.enter_context(tc.tile_pool(name="wh0dp", bufs=2))
            wh1p = ph_l.enter_context(tc.tile_pool(name="wh1p", bufs=2))

            ss0 = psum_ss.tile([1, BC], f32, tag="ss", name="ss_l0")
            for g in range(BLOCKS):
                # stream the resident bf16 deter chunk this block needs
                # (also the mix operand later); overlapped with block g-1
                nc.sync.dma_start(
                    out=dtb_sb[:, 4 * g:4 * g + 4, :],
                    in_=dTb[512 * g:512 * (g + 1), :].rearrange(
                        "(s p) b -> p s b", p=P))
                wx = wh0xp.tile([P, 8, 2, OUT_B], fp8, tag="wh0x",
                                name=f"wh0x_{g}")
                nc.sync.dma_start(out=wx, in_=Wh0x[g])
                if L0DG_BF16:
                    wd = wh0dp.tile([P, 4, OUT_B], bf16, tag="wh0d",
                                    name=f"wh0d_{g}")
                else:
                    wd = wh0dp.tile([P, 2, 2, OUT_B], fp8, tag="wh0d",
                                    name=f"wh0d_{g}")
                nc.sync.dma_start(out=wd, in_=Wh0d[g])
                accs = [psum_acc.tile([P, BC], f32, tag="acc",
                                      name=f"acc_h0_{g}_{m}")
                        for m in range(4)]
                if not L0DG_BF16:
                    raise NotImplementedError(
                        "fp8 L0 dg path needs resident fp8 deter")
                for m in range(4):
                    for kk in range(4):
                        nc.tensor.matmul(
                            accs[m], lhsT=wd[:, kk, m * P:(m + 1) * P],
                            rhs=dtb_sb[:, 4 * g + kk, :],
                            start=(kk == 0), stop=False)
                    for kp in range(8):
                        nc.tensor.matmul(
                            accs[m], lhsT=wx[:, kp, :, m * P:(m + 1) * P],
                            rhs=x8_sb[:, 2 * kp:2 * kp + 2, :],
                            start=False, stop=(kp == 7), perf_mode=DR)
                unit_post(main_sb[:, 4 * g:4 * g + 4, :], accs,
                          C_BH064 + 4 * g, ss0, g == 0, g == BLOCKS - 1,
                          f"l0_{g}")
            invb0 = finish_norm(ss0, DETER, "l0")

            # --------- L1 (bf16), interleaved with the L0 norm ---------
            ss1 = psum_ss.tile([1, BC], f32, tag="ss", name="ss_l1")
            for g in range(BLOCKS):
                unit = main_sb[:, 4 * g:4 * g + 4, :]
                norm_silu_unit(unit, invb0, unit, f"l1_{g}")  # h0n in place
                if L1_BF16:
                    w1h = wh1p.tile([P, 4, OUT_B], bf16, tag="wh1",
                                    name=f"wh1_{g}")
                else:
                    w1h = wh1p.tile([P, 2, 2, OUT_B], fp8, tag="wh1",
                                    name=f"wh1_{g}")
                nc.sync.dma_start(out=w1h, in_=Wh1[g])
                accs = [psum_acc.tile([P, BC], f32, tag="acc",
                                      name=f"acc_h1_{g}_{m}")
                        for m in range(4)]
                for m in range(4):
                    if L1_BF16:
                        for kk in range(4):
                            nc.tensor.matmul(
                                accs[m], lhsT=w1h[:, kk, m * P:(m + 1) * P],
                                rhs=unit[:, kk, :],
                                start=(kk == 0), stop=(kk == 3))
                    else:
                        for kp in range(2):
                            nc.tensor.matmul(
                                accs[m], lhsT=w1h[:, kp, :, m * P:(m + 1) * P],
                                rhs=unit[:, 2 * kp:2 * kp + 2, :],
                                start=(kp == 0), stop=(kp == 1),
                                perf_mode=DR)
                unit_post(unit, accs, C_BH164 + 4 * g, ss1,
                          g == 0, g == BLOCKS - 1, f"l1_{g}")
            invb1 = finish_norm(ss1, DETER, "l1")

        # ------------- GRU gates + final mix (per block) -------------
        with ExitStack() as ph_g:
            wgp = ph_g.enter_context(tc.tile_pool(name="wgpool", bufs=2))
            grup = ph_g.enter_context(tc.tile_pool(name="grup", bufs=2))

            for g in range(BLOCKS):
                unit = main_sb[:, 4 * g:4 * g + 4, :]
                norm_silu_unit(unit, invb1, h1n8[:, 4 * g:4 * g + 4, :],
                               f"g{g}")
                wg = wgp.tile([P, 2, 2, 3 * OUT_B], fp8, tag="wg",
                              name=f"wg_{g}")
                nc.sync.dma_start(out=wg, in_=Wgp[g])
                r_sb = grup.tile([P, 4, BC], bf16, tag="r", name=f"r_{g}")
                c_sb = grup.tile([P, 4, BC], bf16, tag="c", name=f"c_{g}")
                u_sb = grup.tile([P, 4, BC], bf16, tag="u", name=f"u_{g}")
                for mm in range(12):
                    acc = psum_acc.tile([P, BC], f32, tag="acc",
                                        name=f"acc_g{g}_{mm}")
                    for kp in range(2):
                        nc.tensor.matmul(
                            acc, lhsT=wg[:, kp, :, mm * P:(mm + 1) * P],
                            rhs=h1n8[:, 4 * g + 2 * kp:4 * g + 2 * kp + 2, :],
                            start=(kp == 0), stop=(kp == 1), perf_mode=DR)
                    j = 12 * g + mm
                    if mm < 4:
                        nc.scalar.activation(
                            out=r_sb[:, mm, :], in_=acc, func=AF.Sigmoid,
                            bias=bgt[:, j:j + 1], scale=1.0 / WS)
                    elif mm < 8:
                        m = mm - 4
                        nc.gpsimd.scalar_tensor_tensor(
                            out=c_sb[:, m, :], in0=acc,
                            scalar=bg64[:, j:j + 1], in1=r_sb[:, m, :],
                            op0=Alu.add, op1=Alu.mult)
                    else:
                        m = mm - 8
                        nc.scalar.activation(
                            out=u_sb[:, m, :], in_=acc, func=AF.Sigmoid,
                            bias=bgm1[:, j:j + 1], scale=1.0 / WS)
                cflat = c_sb.rearrange("p a b -> p (a b)")
                nc.scalar.activation(out=cflat, in_=cflat, func=AF.Tanh,
                                     scale=1.0 / WS)
                dunit = dtb_sb[:, 4 * g:4 * g + 4, :].rearrange(
                    "p a b -> p (a b)")
                uflat = u_sb.rearrange("p a b -> p (a b)")
                # out = d + u*(c-d), all bf16 on DVE, in place in c_sb
                nc.vector.tensor_sub(cflat, cflat, dunit)
                nc.vector.tensor_mul(cflat, uflat, cflat)
                nc.vector.tensor_add(cflat, dunit, cflat)
                nc.sync.dma_start(
                    out=outT[512 * g:512 * (g + 1), :].rearrange(
                        "(s p) b -> p s b", p=P),
                    in_=c_sb)

    nc.compile()
    return nc


def _get_program():
    global _PROG
    if _PROG is None:
        _PROG = _build_program()
    return _PROG


FP8NP = _ml.float8_e4m3


def _drpack(W, dt):
    """[K, M] -> [K//256, 128, 2, M] DoubleRow-packed, cast to dt."""
    K, M = W.shape
    return np.ascontiguousarray(
        W.reshape(K // 256, 2, P, M).transpose(0, 2, 1, 3)).astype(dt)


def _kpack(W, dt):
    """[K, M] -> [128, K//128, M] (plain k-tiled lhsT), cast to dt."""
    K, M = W.shape
    return np.ascontiguousarray(
        W.reshape(K // P, P, M).transpose(1, 0, 2)).astype(dt)


def _make_const_block(inputs):
    f = lambda a: np.asarray(a, dtype=np.float32)
    cst = np.zeros((P, C_NCOL), dtype=np.float32)
    cst[:, C_BX64:C_BX64 + 16] = WS * np.stack(
        [f(inputs[b]) * f(inputs[g]) for b, g in
         (("b0", "g0"), ("b1", "g1"), ("b2", "g2"), ("b3", "g3"))]
    ).reshape(16, P).T
    cst[:, C_BH064:C_BH064 + 32] = WS * (
        f(inputs["bh0"]) * f(inputs["gh0"])).reshape(32, P).T
    bgt = f(inputs["bg"]).reshape(96, P).T
    cst[:, C_BH164:C_BH164 + 32] = WS * (
        f(inputs["bh1"]) * f(inputs["gh1"])).reshape(32, P).T
    cst[:, C_BG:C_BG + 96] = bgt
    cst[:, C_BG64:C_BG64 + 96] = WS * bgt
    cst[:, C_BGM1:C_BGM1 + 96] = bgt - 1.0
    cst[:, C_EPSH] = WS * WS * EPS
    return cst


def _prep_inputs(inputs):
    """Host-side shard + transpose + quantized weight packing."""
    f = lambda a: np.ascontiguousarray(np.asarray(a), dtype=np.float32)
    stoch = f(inputs["stoch"]).reshape(B, -1)
    deter = f(inputs["deter"])
    action = f(inputs["action"])
    d_emb = f(inputs["d_emb"])

    g0, g1 = f(inputs["g0"]), f(inputs["g1"])
    g2, g3 = f(inputs["g2"]), f(inputs["g3"])
    gh0, gh1 = f(inputs["gh0"]), f(inputs["gh1"])

    W0 = WS * f(inputs["W0"]) * g0
    W1 = WS * f(inputs["W1"]) * g1
    Wh0 = WS * f(inputs["Wh0"]) * gh0.reshape(BLOCKS, 1, OUT_B)
    Wh1 = WS * f(inputs["Wh1"]) * gh1.reshape(BLOCKS, 1, OUT_B)
    Wg = WS * f(inputs["Wg"])

    l0dg_dt = _ml.bfloat16 if L0DG_BF16 else FP8NP
    l1_dt = _ml.bfloat16 if L1_BF16 else FP8NP
    if L0DG_BF16:
        wh0d = np.stack([_kpack(Wh0[g, :OUT_B], _ml.bfloat16)
                         for g in range(BLOCKS)])
    else:
        wh0d = np.stack([_drpack(Wh0[g, :OUT_B], FP8NP)
                         for g in range(BLOCKS)])
    if L1_BF16:
        wh1 = np.stack([_kpack(Wh1[g], _ml.bfloat16) for g in range(BLOCKS)])
    else:
        wh1 = np.stack([_drpack(Wh1[g], FP8NP) for g in range(BLOCKS)])

    cst8 = np.ones((P, 2), dtype=FP8NP)
    shared = {
        "W0p": _drpack(W0, FP8NP),
        "W1p": np.ascontiguousarray(
            _drpack(W1, FP8NP).transpose(1, 0, 2, 3)),  # [P, 4, 2, M]
        "W2": (WS * f(inputs["W2"]) * g2).astype(np.float32),
        "W3": (WS * f(inputs["W3"]) * g3).astype(np.float32),
        "Wh0x": np.stack([_drpack(Wh0[g, OUT_B:], FP8NP)
                          for g in range(BLOCKS)]),
        "Wh0d": wh0d,
        "Wh1": wh1,
        "Wgp": np.stack([_drpack(Wg[g], FP8NP) for g in range(BLOCKS)]),
        "cst": _make_const_block(inputs),
        "cst8": cst8,
    }
    # Wh0x packed as [B, pairs, P, 2, M] -> want [B, P, pairs, 2, M]
    shared["Wh0x"] = np.ascontiguousarray(
        shared["Wh0x"].transpose(0, 2, 1, 3, 4))
    shared["Wgp"] = np.ascontiguousarray(
        shared["Wgp"].transpose(0, 2, 1, 3, 4))
    if not L0DG_BF16:
        shared["Wh0d"] = np.ascontiguousarray(
            shared["Wh0d"].transpose(0, 2, 1, 3, 4))
    if not L1_BF16:
        shared["Wh1"] = np.ascontiguousarray(
            shared["Wh1"].transpose(0, 2, 1, 3, 4))
    # W0p stays [16, P, 2, M] (indexed by pair in the DMA loop)

    in_maps = []
    for c in range(NCORES):
        sl = slice(c * BC, (c + 1) * BC)
        m = dict(shared)
        dT = np.ascontiguousarray(deter[sl].T)
        m["dT8"] = dT.astype(FP8NP)
        m["dTb"] = dT.astype(_ml.bfloat16)
        m["sT8"] = np.ascontiguousarray(stoch[sl].T).astype(FP8NP)
        m["aT"] = np.ascontiguousarray(action[sl].T)
        m["eT"] = np.ascontiguousarray(d_emb[sl].T)
        in_maps.append(m)
    return in_maps


def _run(inputs, trace=False):
    from concourse import bass_utils
    nc = _get_program()
    in_maps = _prep_inputs(inputs)
    res = bass_utils.run_bass_kernel_spmd(
        nc, in_maps, core_ids=list(range(NCORES)), trace=trace)
    out = np.empty((B, DETER), dtype=np.float32)
    for c in range(NCORES):
        out[c * BC:(c + 1) * BC, :] = \
            np.asarray(res.results[c]["outT"]).astype(np.float32).T
    return out, res.exec_time_ns


def kernel(**inputs):
    out, _ = _run(inputs, trace=False)
    return out


# ---------------------------------------------------------------------------
# benchmarking helper (test-only; the grading path is kernel() above)
# ---------------------------------------------------------------------------

def _bench_generic(nc, in_maps, iters, n_cores=None):
    """Time repeated device executions with device-resident inputs."""
    import time
    import jax
    import concourse.mybir as mybir
    from jax.sharding import Mesh, NamedSharding, PartitionSpec
    from jax.experimental.shard_map import shard_map
    from concourse import bass2jax

    bass2jax.install_neuronx_cc_hook()
    if n_cores is None:
        n_cores = len(in_maps)

    in_names, out_names, out_avals = [], [], []
    for alloc in nc.m.functions[0].allocations:
        if not isinstance(alloc, mybir.MemoryLocationSet):
            continue
        name = alloc.memorylocations[0].name
        pid_name = (nc.partition_id_tensor.name
                    if nc.partition_id_tensor else None)
        if alloc.kind == "ExternalInput":
            if name != pid_name:
                in_names.append(name)
        elif alloc.kind == "ExternalOutput":
            out_names.append(name)
            out_avals.append(jax.core.ShapedArray(
                tuple(alloc.tensor_shape), mybir.dt.np(alloc.dtype)))
    n_params = len(in_names)

    pid_name = nc.partition_id_tensor.name if nc.partition_id_tensor else None
    bind_names = in_names + out_names + ([pid_name] if pid_name else [])

    def _body(*args):
        operands = list(args)
        if pid_name:
            operands.append(bass2jax.partition_id_tensor())
        outs = bass2jax._bass_exec_p.bind(
            *operands,
            out_avals=tuple(out_avals),
            in_names=tuple(bind_names),
            out_names=tuple(out_names),
            lowering_input_output_aliases=(),
            sim_require_finite=True,
            sim_require_nnan=True,
            nc=nc,
        )
        return tuple(outs)

    devices = jax.devices()[:n_cores]
    mesh = Mesh(np.asarray(devices), ("core",))
    nshard = NamedSharding(mesh, PartitionSpec("core"))
    sharded = jax.jit(
        shard_map(_body, mesh=mesh,
                  in_specs=(PartitionSpec("core"),) * (n_params + len(out_names)),
                  out_specs=(PartitionSpec("core"),) * len(out_names),
                  check_rep=False),
        keep_unused=True)

    concat_in = [
        jax.device_put(
            np.concatenate([np.asarray(in_maps[c][nm]) for c in range(n_cores)],
                           axis=0), nshard)
        for nm in in_names]
    concat_zeros = [
        jax.device_put(
            np.zeros((n_cores * a.shape[0], *a.shape[1:]), a.dtype), nshard)
        for a in out_avals]

    outs = sharded(*concat_in, *concat_zeros)
    jax.block_until_ready(outs)

    BATCH = 6
    diffs = []
    for _ in range(iters):
        t0 = time.perf_counter()
        outs = sharded(*concat_in, *concat_zeros)
        jax.block_until_ready(outs)
        t1 = time.perf_counter()
        for _ in range(BATCH):
            outs = sharded(*concat_in, *concat_zeros)
        jax.block_until_ready(outs)
        t2 = time.perf_counter()
        diffs.append((t2 - t1) - (t1 - t0))
    diffs.sort()
    per_iter_ns = diffs[len(diffs) // 2] / (BATCH - 1) * 1e9
    return outs, per_iter_ns


_TINY = None


def _tiny_program():
    """Near-noop program with the SAME input/output signature, to measure
    axon dispatch overhead differentially."""
    global _TINY
    if _TINY is None:
        nc = bacc.Bacc(trn_type="TRN2", target_bir_lowering=False, debug=False)
        d = {"dT8": ((DETER, BC), fp8), "dTb": ((DETER, BC), bf16),
             "sT8": ((STOCH, BC), fp8), "aT": ((ACT_DIM, BC), f32),
             "eT": ((DEMB, BC), f32), "W0p": ((16, P, 2, HIDDEN), fp8),
             "W1p": ((P, 4, 2, HIDDEN), fp8), "W2": ((ACT_DIM, HIDDEN), f32),
             "W3": ((DEMB, HIDDEN), f32),
             "Wh0x": ((BLOCKS, P, 8, 2, OUT_B), fp8),
             "Wh0d": ((BLOCKS, P, 4, OUT_B), bf16) if L0DG_BF16
             else ((BLOCKS, P, 2, 2, OUT_B), fp8),
             "Wh1": ((BLOCKS, P, 4, OUT_B), bf16) if L1_BF16
             else ((BLOCKS, P, 2, 2, OUT_B), fp8),
             "Wgp": ((BLOCKS, P, 2, 2, 3 * OUT_B), fp8),
             "cst": ((P, C_NCOL), f32), "cst8": ((P, 2), fp8)}
        aps = {k: nc.dram_tensor(k, list(s), dt, kind="ExternalInput").ap()
               for k, (s, dt) in d.items()}
        outT = nc.dram_tensor("outT", [DETER, BC], bf16,
                              kind="ExternalOutput").ap()
        with tile.TileContext(nc) as tc:
            with tc.tile_pool(name="t", bufs=2) as pool:
                t = pool.tile([P, 4, BC], bf16)
                nc.sync.dma_start(
                    out=t, in_=aps["dTb"][:512, :].rearrange(
                        "(s p) b -> p s b", p=P))
                for g in range(BLOCKS):
                    nc.sync.dma_start(
                        out=outT[512 * g:512 * (g + 1), :].rearrange(
                            "(s p) b -> p s b", p=P),
                        in_=t)
        nc.compile()
        _TINY = nc
    return _TINY


def _bench_overhead(inputs, iters=20):
    nc = _tiny_program()
    in_maps = _prep_inputs(inputs)
    _, t = _bench_generic(nc, in_maps, iters)
    return t


def _bench(inputs, iters=20):
    nc = _get_program()
    in_maps = _prep_inputs(inputs)
    outs, per_iter_ns = _bench_generic(nc, in_maps, iters)
    res = np.asarray(outs[0]).reshape(NCORES, DETER, BC)
    out = np.empty((B, DETER), dtype=np.float32)
    for c in range(NCORES):
        out[c * BC:(c + 1) * BC, :] = res[c].astype(np.float32).T
    return out, per_iter_ns


# revision 13
# speedup vs baseline: 1.7030x; 1.1281x over previous
"""Trainium2 Bass kernel for the Deter GRU-MLP block (RSSM deter update).

Sharding: data-parallel over batch B=4096 across 8 NeuronCores (512 rows
each), all parameters replicated; no collectives.

Design (v2, mixed fp8/bf16):
- Activations live transposed in SBUF (features on partitions, batch on the
  512-wide free axis); weights consumed in natural [K, M] layout.
- Big GEMMs run as fp8(e4m3) DoubleRow matmuls (0.5 cyc/row, 2 k-tiles per
  instruction): branch W0/W1, the x-part of the block-diagonal L0, and the
  GRU gate GEMM.  The deter slice of L0 and all of L1 stay bf16 for
  accuracy (numpy-simulated end-to-end rel-max err ~1.2e-2 vs 2e-2 budget).
- All weights are scaled by 64 on the host (fp8 normal range) with norm
  gains folded in; pre-norm PSUM results are therefore 64*y.  The rmsnorm
  absorbs the 64: squares are rescaled to true scale on the Pool engine
  ((y64/4096)*y64) and the 64 folds into the reciprocal-rms; the gate
  nonlinearities use the ACT scale operand (sigmoid(acc/64 + b)).
- Pre-norm y64 is stored bf16; squares are written fp8 so the ones-vector
  sum-of-squares matmuls also run DoubleRow.  silu runs as one batched ACT
  op per 4-tile norm unit, casting to fp8 (or bf16 for the L1 input) on
  write.
- deter is resident in SBUF as bf16 (L0 dg slice + final GRU mix operand);
  a second fp8 copy streams through once for the branch-0 GEMM.  Output is
  written bf16 and upcast on the host.
"""

import os
import sys
from contextlib import ExitStack

import numpy as np
import ml_dtypes as _ml

for _p in ("/opt/trn_rl_repo", "/opt/pypackages"):
    if os.path.isdir(_p) and _p not in sys.path:
        sys.path.insert(0, _p)

os.environ.setdefault("MYCRO_LOCAL_CACHE", "1")

import concourse.bass as bass  # noqa: E402
import concourse.bacc as bacc  # noqa: E402
import concourse.mybir as mybir  # noqa: E402
import concourse.tile as tile  # noqa: E402

# ---- problem constants (hardcoded; kernel.py must be self-contained) ----
P = 128
B = 4096
NCORES = 8
BC = B // NCORES  # 512 batch columns per core
DETER = 4096
STOCH = 1024
ACT_DIM = 32
DEMB = 16
HIDDEN = 512
BLOCKS = 8
OUT_B = DETER // BLOCKS  # 512
IN_B0 = 4 * HIDDEN + OUT_B  # 2560
EPS = 1e-4

ND = DETER // P       # 32 deter k/n tiles
NX = 4 * HIDDEN // P  # 16 x k tiles
WS = 64.0             # weight scale folded into rmsnorm / gate scales

# const-block column layout (single [P, C_NCOL] f32 DRAM input)
C_BX64 = 0            # 16: 64*(branch bias * gain)
C_BH064 = 16          # 32: 64*(bh0 * gh0)
C_BH164 = 48          # 32: 64*(bh1 * gh1)
C_BG = 80             # 96: bg (unscaled, sigmoid bias)
C_BG64 = 176          # 96: 64*bg (cand stt bias)
C_BGM1 = 272          # 96: bg - 1 (update sigmoid bias)
C_EPSH = 368          # 1: 4096*EPS
C_NEG1 = 369          # 1: -1.0 (update-gate bias)
C_NCOL = 370

f32 = mybir.dt.float32
f32r = mybir.dt.float32r
bf16 = mybir.dt.bfloat16
fp8 = mybir.dt.float8e4

# precision flags (see fp8_sim2.py): bf16 for the L0 deter slice and L1
L0DG_BF16 = True
L1_BF16 = True

_PROG = None


def _r(ap):
    return ap.bitcast(f32r)


def _build_program():
    """Build the single-core SPMD Bass program (same on all 8 cores)."""
    AF = mybir.ActivationFunctionType
    Alu = mybir.AluOpType
    DR = mybir.MatmulPerfMode.DoubleRow
    nc = bacc.Bacc(trn_type="TRN2", target_bir_lowering=False, debug=False)

    def din(name, shape, dt=f32):
        return nc.dram_tensor(name, list(shape), dt, kind="ExternalInput").ap()

    dT8 = din("dT8", (DETER, BC), fp8)
    dTb = din("dTb", (DETER, BC), bf16)
    sT8 = din("sT8", (STOCH, BC), fp8)
    aT = din("aT", (ACT_DIM, BC))
    eT = din("eT", (DEMB, BC))
    W0p = din("W0p", (16, P, 2, HIDDEN), fp8)
    W1p = din("W1p", (P, 4, 2, HIDDEN), fp8)
    W2 = din("W2", (ACT_DIM, HIDDEN))
    W3 = din("W3", (DEMB, HIDDEN))
    Wh0x = din("Wh0x", (BLOCKS, P, 8, 2, OUT_B), fp8)
    Wh0d = din("Wh0d", (BLOCKS, P, 4, OUT_B), bf16)
    Wh1 = din("Wh1", (BLOCKS, P, 4, OUT_B), bf16)
    Wgp = din("Wgp", (BLOCKS, P, 3, 2, 3 * OUT_B), fp8)
    cst = din("cst", (P, C_NCOL))
    cst8 = din("cst8", (P, 2 + 2 * BC), fp8)
    outT = nc.dram_tensor("outT", [DETER, BC], bf16,
                          kind="ExternalOutput").ap()

    with tile.TileContext(nc) as tc, ExitStack() as top:
        consts = top.enter_context(tc.tile_pool(name="consts", bufs=1))
        cst_sb = consts.tile([P, C_NCOL], f32)
        nc.sync.dma_start(out=_r(cst_sb), in_=_r(cst))
        cst8_sb = consts.tile([P, 2 + 2 * BC], fp8)
        nc.sync.dma_start(out=cst8_sb, in_=cst8)
        ones8 = cst8_sb[:, 0:2]
        onesp = cst8_sb[:, 2:2 + 2 * BC].rearrange("p (j b) -> p j b", j=2)
        bgm1 = cst_sb[:, C_BGM1:C_BGM1 + 96]
        epsh = cst_sb[:1, C_EPSH:C_EPSH + 1]
        neg1 = cst_sb[:, C_NEG1:C_NEG1 + 1]

        # resident regions
        mainp = top.enter_context(tc.tile_pool(name="mainp", bufs=1))
        main_sb = mainp.tile([P, ND, BC], bf16)   # pre-norm y64 / h0n
        dtbp = top.enter_context(tc.tile_pool(name="dtbp", bufs=1))
        dtb_sb = dtbp.tile([P, ND, BC], bf16)     # deter bf16 (L0 dg + mix)
        x8p = top.enter_context(tc.tile_pool(name="x8p", bufs=1))
        x8_sb = x8p.tile([P, NX, BC], fp8)        # branch outputs (L0 rhs)
        h1p = top.enter_context(tc.tile_pool(name="h1p", bufs=1))
        h1n8 = h1p.tile([P, ND, BC], fp8)         # L1 normalized (gates rhs)

        ysqp = top.enter_context(tc.tile_pool(name="ysqp", bufs=2))
        invp = top.enter_context(tc.tile_pool(name="invp", bufs=2))
        invbp = top.enter_context(tc.tile_pool(name="invbp", bufs=2))
        sigp = top.enter_context(tc.tile_pool(name="sigp", bufs=2))

        def act_warm(func, name):
            """Trigger an ACT table switch off the critical path."""
            t = invp.tile([1, 1], f32, tag="warm", name=f"warm_{name}")
            nc.scalar.activation(out=t, in_=epsh, func=func)

        def finish_norm(ss_flat, D, width, name):
            """invb = 1/(64*sqrt(ss/D + eps)), bf16, broadcast to all
            partitions.  ss_flat: [1, width*BC] (PSUM)."""
            sq = invp.tile([1, width * BC], f32, tag="sq", name=f"sq_{name}")
            nc.scalar.activation(out=sq, in_=ss_flat, func=AF.Sqrt,
                                 bias=epsh, scale=(WS * WS) / D)
            act_warm(AF.Sigmoid, f"sg_{name}")  # reload hidden under recip
            inv1 = invp.tile([1, width * BC], bf16, tag="inv1",
                             name=f"inv1_{name}")
            with nc.allow_low_precision(reason="bf16 rstd broadcast"):
                nc.vector.reciprocal(inv1, sq)
            invb = invbp.tile([P, width * BC], bf16, tag="invb",
                              name=f"invb_{name}")
            nc.gpsimd.partition_broadcast(invb, inv1)
            return invb

        def norm_silu_unit(unit_y, invb, out_unit, name, per_tile=False):
            """out_unit <- silu(unit_y * invb) = t*sigmoid(t), t=y*inv.
            per_tile=True pipelines at tile granularity (lower latency
            right after a norm barrier)."""
            for m in range(4):
                nc.vector.tensor_mul(unit_y[:, m, :], unit_y[:, m, :],
                                     invb[:, m * BC:(m + 1) * BC]
                                     if invb.shape[-1] == 4 * BC else invb)
            s = sigp.tile([P, 4, BC], bf16, tag="sig", name=f"sig_{name}")
            if per_tile:
                for m in range(4):
                    nc.scalar.activation(out=s[:, m, :], in_=unit_y[:, m, :],
                                         func=AF.Sigmoid)
                    nc.vector.tensor_mul(out_unit[:, m, :], unit_y[:, m, :],
                                         s[:, m, :])
            else:
                nc.scalar.activation(
                    out=s.rearrange("p a b -> p (a b)"),
                    in_=unit_y.rearrange("p a b -> p (a b)"), func=AF.Sigmoid)
                nc.vector.tensor_mul(
                    out_unit.rearrange("p a b -> p (a b)"),
                    unit_y.rearrange("p a b -> p (a b)"),
                    s.rearrange("p a b -> p (a b)"))

        # ============ phases A, L0, L1 (shared PSUM layout) ============
        with ExitStack() as ph_al:
            psum_acc = ph_al.enter_context(
                tc.tile_pool(name="pacc", bufs=4, space="PSUM"))
            psum_ss = ph_al.enter_context(
                tc.tile_pool(name="pss", bufs=1, space="PSUM"))

            def unit_post(unit_y, accs, b64col0, ss, ss_first, ss_last,
                          name, act_copy):
                """copy accs (+64*bias) into unit_y (bf16) -- on ACT
                (Identity) when act_copy else DVE; squares at true scale
                (Pool, fp8); DoubleRow ones-reduction into ss."""
                for m in range(4):
                    bcol = cst_sb[:, b64col0 + m:b64col0 + m + 1]
                    if act_copy:
                        nc.scalar.activation(out=unit_y[:, m, :],
                                             in_=accs[m], func=AF.Identity,
                                             bias=bcol)
                    else:
                        nc.vector.tensor_scalar_add(unit_y[:, m, :],
                                                    accs[m], bcol)
                ysq = ysqp.tile([P, 4, BC], fp8, tag="ysq",
                                name=f"ysq_{name}")
                for half in range(2):
                    seg = unit_y[:, 2 * half:2 * half + 2, :]
                    nc.gpsimd.scalar_tensor_tensor(
                        out=ysq[:, 2 * half:2 * half + 2, :].rearrange(
                            "p a b -> p (a b)"),
                        in0=seg.rearrange("p a b -> p (a b)"),
                        scalar=1.0 / (WS * WS),
                        in1=seg.rearrange("p a b -> p (a b)"),
                        op0=Alu.mult, op1=Alu.mult)
                    nc.tensor.matmul(
                        ss, lhsT=ones8, rhs=ysq[:, 2 * half:2 * half + 2, :],
                        start=(ss_first and half == 0),
                        stop=(ss_last and half == 1), perf_mode=DR)

            # ---------------- phase A: four input branches ----------------
            with ExitStack() as ph_a:
                w0p_pool = ph_a.enter_context(
                    tc.tile_pool(name="w0p", bufs=8))
                d8p = ph_a.enter_context(tc.tile_pool(name="d8p", bufs=8))
                sp = ph_a.enter_context(tc.tile_pool(name="sp", bufs=1))

                sT_sb = sp.tile([P, STOCH // P, BC], fp8)
                aT_sb = sp.tile([ACT_DIM, BC], f32)
                eT_sb = sp.tile([DEMB, BC], f32)
                an_sb = sp.tile([ACT_DIM, BC], f32)
                w3t = sp.tile([DEMB, HIDDEN], f32)
                w2t = sp.tile([ACT_DIM, HIDDEN], f32)
                w1t = sp.tile([P, 4, 2, HIDDEN], fp8)

                nc.sync.dma_start(out=_r(eT_sb), in_=_r(eT))
                nc.sync.dma_start(out=_r(w3t), in_=_r(W3))
                nc.sync.dma_start(out=aT_sb, in_=aT)
                nc.sync.dma_start(out=_r(w2t), in_=_r(W2))
                nc.sync.dma_start(out=sT_sb,
                                  in_=sT8.rearrange("(s p) b -> p s b", p=P))
                nc.sync.dma_start(out=w1t, in_=W1p)
                d8_chunks = []
                w0_slabs = []
                for c in range(8):
                    d8 = d8p.tile([P, 4, BC], fp8, tag="d8", name=f"d8_{c}")
                    nc.sync.dma_start(
                        out=d8, in_=dT8[512 * c:512 * (c + 1), :].rearrange(
                            "(s p) b -> p s b", p=P))
                    d8_chunks.append(d8)
                    w0 = w0p_pool.tile([P, 2, 2, HIDDEN], fp8, tag="w0",
                                       name=f"w0_{c}")
                    nc.sync.dma_start(
                        out=w0, in_=W0p[2 * c:2 * c + 2].rearrange(
                            "s p j m -> p s j m"))
                    w0_slabs.append(w0)

                # action preprocess: a / max(|a|, 1)
                ab = sp.tile([ACT_DIM, BC], f32)
                nc.scalar.activation(out=ab, in_=aT_sb, func=AF.Abs)
                nc.vector.tensor_scalar_max(ab, ab, 1.0)
                nc.vector.reciprocal(ab, ab)
                nc.vector.tensor_mul(_r(an_sb), aT_sb, ab)

                def branch_small(br, wt, rhs):
                    accs = []
                    for m in range(4):
                        acc = psum_acc.tile([P, BC], f32, tag="acc",
                                            name=f"acc_br{br}_{m}")
                        nc.tensor.matmul(acc,
                                         lhsT=_r(wt[:, m * P:(m + 1) * P]),
                                         rhs=_r(rhs), start=True, stop=True)
                        accs.append(acc)
                    return accs

                def branch_dr(br, npairs, wslab, rhs_pair):
                    accs = [psum_acc.tile([P, BC], f32, tag="acc",
                                          name=f"acc_br{br}_{m}")
                            for m in range(4)]
                    for kp in range(npairs):
                        w = wslab(kp)
                        rhs = rhs_pair(kp)
                        for m in range(4):
                            nc.tensor.matmul(
                                accs[m], lhsT=w[:, :, m * P:(m + 1) * P],
                                rhs=rhs, start=(kp == 0),
                                stop=(kp == npairs - 1), perf_mode=DR)
                    return accs

                # fused branch norms: one ss slot per branch in a 4-bank
                # PSUM tile -> single sqrt/recip/broadcast for all four
                ss4 = psum_ss.tile([1, 4, BC], f32, tag="ss4", name="ss4")

                def branch_post(br, accs):
                    unit = main_sb[:, 4 * br:4 * br + 4, :]
                    unit_post(unit, accs, C_BX64 + 4 * br, ss4[:, br, :],
                              True, True, f"br{br}", act_copy=False)

                branch_post(3, branch_small(3, w3t, eT_sb))
                branch_post(2, branch_small(2, w2t, an_sb))
                branch_post(1, branch_dr(
                    1, 4, lambda kp: w1t[:, kp, :, :],
                    lambda kp: sT_sb[:, 2 * kp:2 * kp + 2, :]))
                branch_post(0, branch_dr(
                    0, 16, lambda kp: w0_slabs[kp // 2][:, kp % 2, :, :],
                    lambda kp: d8_chunks[kp // 2][
                        :, 2 * (kp % 2):2 * (kp % 2) + 2, :]))

                invb4 = finish_norm(ss4.rearrange("o a b -> o (a b)"),
                                    HIDDEN, 4, "br")
                for br in range(4):
                    norm_silu_unit(
                        main_sb[:, 4 * br:4 * br + 4, :],
                        invb4[:, br * BC:(br + 1) * BC],
                        x8_sb[:, 4 * br:4 * br + 4, :], f"br{br}",
                        per_tile=(br == 0))

            # ---------- L0: BlockLinear(2560 -> 512/block) ----------
            with ExitStack() as ph_l:
                wh0xp = ph_l.enter_context(
                    tc.tile_pool(name="wh0xp", bufs=2))
                wh0dp = ph_l.enter_context(
                    tc.tile_pool(name="wh0dp", bufs=2))
                wh1p = ph_l.enter_context(tc.tile_pool(name="wh1p", bufs=3))

                ss0 = psum_ss.tile([1, BC], f32, tag="ss4", name="ss_l0")
                for g in range(BLOCKS):
                    nc.sync.dma_start(
                        out=dtb_sb[:, 4 * g:4 * g + 4, :],
                        in_=dTb[512 * g:512 * (g + 1), :].rearrange(
                            "(s p) b -> p s b", p=P))
                    wx = wh0xp.tile([P, 8, 2, OUT_B], fp8, tag="wh0x",
                                    name=f"wh0x_{g}")
                    nc.sync.dma_start(out=wx, in_=Wh0x[g])
                    wd = wh0dp.tile([P, 4, OUT_B], bf16, tag="wh0d",
                                    name=f"wh0d_{g}")
                    nc.sync.dma_start(out=wd, in_=Wh0d[g])
                    accs = [psum_acc.tile([P, BC], f32, tag="acc",
                                          name=f"acc_h0_{g}_{m}")
                            for m in range(4)]
                    for m in range(4):
                        for kk in range(4):
                            nc.tensor.matmul(
                                accs[m], lhsT=wd[:, kk, m * P:(m + 1) * P],
                                rhs=dtb_sb[:, 4 * g + kk, :],
                                start=(kk == 0), stop=False)
                        for kp in range(8):
                            nc.tensor.matmul(
                                accs[m], lhsT=wx[:, kp, :, m * P:(m + 1) * P],
                                rhs=x8_sb[:, 2 * kp:2 * kp + 2, :],
                                start=False, stop=(kp == 7), perf_mode=DR)
                    unit_post(main_sb[:, 4 * g:4 * g + 4, :], accs,
                              C_BH064 + 4 * g, ss0, g == 0, g == BLOCKS - 1,
                              f"l0_{g}", act_copy=True)
                    if g == 5:
                        act_warm(AF.Sqrt, "l0")
                invb0 = finish_norm(ss0, DETER, 1, "l0")

                # --------- L1 (bf16), interleaved with the L0 norm ---------
                ss1 = psum_ss.tile([1, BC], f32, tag="ss4", name="ss_l1")
                for g in range(BLOCKS):
                    unit = main_sb[:, 4 * g:4 * g + 4, :]
                    norm_silu_unit(unit, invb0, unit, f"l1_{g}",
                                   per_tile=(g == 0))  # h0n in place
                    w1h = wh1p.tile([P, 4, OUT_B], bf16, tag="wh1",
                                    name=f"wh1_{g}")
                    nc.sync.dma_start(out=w1h, in_=Wh1[g])
                    accs = [psum_acc.tile([P, BC], f32, tag="acc",
                                          name=f"acc_h1_{g}_{m}")
                            for m in range(4)]
                    for m in range(4):
                        for kk in range(4):
                            nc.tensor.matmul(
                                accs[m], lhsT=w1h[:, kk, m * P:(m + 1) * P],
                                rhs=unit[:, kk, :],
                                start=(kk == 0), stop=(kk == 3))
                    unit_post(unit, accs, C_BH164 + 4 * g, ss1,
                              g == 0, g == BLOCKS - 1, f"l1_{g}",
                              act_copy=True)
                    if g == 5:
                        act_warm(AF.Sqrt, "l1")
                invb1 = finish_norm(ss1, DETER, 1, "l1")

        # ------------- GRU gates + final mix (per block) -------------
        with ExitStack() as ph_g:
            gpsum = ph_g.enter_context(
                tc.tile_pool(name="gpsum", bufs=2, space="PSUM"))
            wgp = ph_g.enter_context(tc.tile_pool(name="wgpool", bufs=3))
            grup = ph_g.enter_context(tc.tile_pool(name="grup", bufs=2))

            for g in range(BLOCKS):
                unit = main_sb[:, 4 * g:4 * g + 4, :]
                norm_silu_unit(unit, invb1, h1n8[:, 4 * g:4 * g + 4, :],
                               f"g{g}", per_tile=(g == 0))
                wg = wgp.tile([P, 3, 2, 3 * OUT_B], fp8, tag="wg",
                              name=f"wg_{g}")
                nc.sync.dma_start(out=wg, in_=Wgp[g])
                r_sb = grup.tile([P, 4, BC], bf16, tag="r", name=f"r_{g}")
                c_sb = grup.tile([P, 4, BC], bf16, tag="c", name=f"c_{g}")
                u_sb = grup.tile([P, 4, BC], bf16, tag="u", name=f"u_{g}")

                def gate_group(grp):
                    """12 DoubleRow matmuls (2 data pairs + bias pair) into
                    a 4-bank PSUM group for gate third grp."""
                    acc4 = gpsum.tile([P, 4, BC], f32, tag="g4",
                                      name=f"acc_g{g}_{grp}")
                    for m in range(4):
                        mm = 4 * grp + m
                        for kp in range(3):
                            rhs = (onesp if kp == 2 else
                                   h1n8[:, 4 * g + 2 * kp:4 * g + 2 * kp + 2, :])
                            nc.tensor.matmul(
                                acc4[:, m, :],
                                lhsT=wg[:, kp, :, mm * P:(mm + 1) * P],
                                rhs=rhs, start=(kp == 0), stop=(kp == 2),
                                perf_mode=DR)
                    return acc4

                accr = gate_group(0)
                nc.scalar.activation(
                    out=r_sb.rearrange("p a b -> p (a b)"),
                    in_=accr.rearrange("p a b -> p (a b)"),
                    func=AF.Sigmoid, scale=1.0 / WS)
                accc = gate_group(1)
                nc.gpsimd.scalar_tensor_tensor(
                    out=c_sb.rearrange("p a b -> p (a b)"),
                    in0=accc.rearrange("p a b -> p (a b)"),
                    scalar=1.0 / WS,
                    in1=r_sb.rearrange("p a b -> p (a b)"),
                    op0=Alu.mult, op1=Alu.mult)
                nc.scalar.activation(
                    out=c_sb.rearrange("p a b -> p (a b)"),
                    in_=c_sb.rearrange("p a b -> p (a b)"), func=AF.Tanh)
                accu = gate_group(2)
                nc.scalar.activation(
                    out=u_sb.rearrange("p a b -> p (a b)"),
                    in_=accu.rearrange("p a b -> p (a b)"),
                    func=AF.Sigmoid, scale=1.0 / WS, bias=neg1)

                cflat = c_sb.rearrange("p a b -> p (a b)")
                dunit = dtb_sb[:, 4 * g:4 * g + 4, :].rearrange(
                    "p a b -> p (a b)")
                uflat = u_sb.rearrange("p a b -> p (a b)")
                # out = d + u*(c-d), all bf16 on DVE, in place in c_sb
                nc.vector.tensor_sub(cflat, cflat, dunit)
                nc.vector.tensor_mul(cflat, uflat, cflat)
                nc.vector.tensor_add(cflat, dunit, cflat)
                nc.sync.dma_start(
                    out=outT[512 * g:512 * (g + 1), :].rearrange(
                        "(s p) b -> p s b", p=P),
                    in_=c_sb)

    nc.compile()
    return nc


def _get_program():
    global _PROG
    if _PROG is None:
        _PROG = _build_program()
    return _PROG


FP8NP = _ml.float8_e4m3


def _drpack(W, dt):
    """[K, M] -> [K//256, 128, 2, M] DoubleRow-packed, cast to dt."""
    K, M = W.shape
    return np.ascontiguousarray(
        W.reshape(K // 256, 2, P, M).transpose(0, 2, 1, 3)).astype(dt)


def _kpack(W, dt):
    """[K, M] -> [128, K//128, M] (plain k-tiled lhsT), cast to dt."""
    K, M = W.shape
    return np.ascontiguousarray(
        W.reshape(K // P, P, M).transpose(1, 0, 2)).astype(dt)


def _make_const_block(inputs):
    f = lambda a: np.asarray(a, dtype=np.float32)
    cst = np.zeros((P, C_NCOL), dtype=np.float32)
    cst[:, C_BX64:C_BX64 + 16] = WS * np.stack(
        [f(inputs[b]) * f(inputs[g]) for b, g in
         (("b0", "g0"), ("b1", "g1"), ("b2", "g2"), ("b3", "g3"))]
    ).reshape(16, P).T
    cst[:, C_BH064:C_BH064 + 32] = WS * (
        f(inputs["bh0"]) * f(inputs["gh0"])).reshape(32, P).T
    bgt = f(inputs["bg"]).reshape(96, P).T
    cst[:, C_BH164:C_BH164 + 32] = WS * (
        f(inputs["bh1"]) * f(inputs["gh1"])).reshape(32, P).T
    cst[:, C_BG:C_BG + 96] = bgt
    cst[:, C_BG64:C_BG64 + 96] = WS * bgt
    cst[:, C_BGM1:C_BGM1 + 96] = bgt - 1.0
    cst[:, C_EPSH] = WS * WS * EPS
    cst[:, C_NEG1] = -1.0
    return cst


def _prep_inputs(inputs):
    """Host-side shard + transpose + quantized weight packing."""
    f = lambda a: np.ascontiguousarray(np.asarray(a), dtype=np.float32)
    stoch = f(inputs["stoch"]).reshape(B, -1)
    deter = f(inputs["deter"])
    action = f(inputs["action"])
    d_emb = f(inputs["d_emb"])

    g0, g1 = f(inputs["g0"]), f(inputs["g1"])
    g2, g3 = f(inputs["g2"]), f(inputs["g3"])
    gh0, gh1 = f(inputs["gh0"]), f(inputs["gh1"])

    W0 = WS * f(inputs["W0"]) * g0
    W1 = WS * f(inputs["W1"]) * g1
    Wh0 = WS * f(inputs["Wh0"]) * gh0.reshape(BLOCKS, 1, OUT_B)
    Wh1 = WS * f(inputs["Wh1"]) * gh1.reshape(BLOCKS, 1, OUT_B)
    Wg = WS * f(inputs["Wg"])

    wh0d = np.stack([_kpack(Wh0[g, :OUT_B], _ml.bfloat16)
                     for g in range(BLOCKS)])
    wh1 = np.stack([_kpack(Wh1[g], _ml.bfloat16) for g in range(BLOCKS)])

    bg = f(inputs["bg"])  # [3*DETER], block g segment [1536g:1536(g+1)]
    wgp = np.zeros((BLOCKS, 3, P, 2, 3 * OUT_B), dtype=FP8NP)
    for g in range(BLOCKS):
        wgp[g, :2] = _drpack(Wg[g], FP8NP)
        wgp[g, 2, 0, 0, :] = (WS * bg[1536 * g:1536 * (g + 1)]).astype(FP8NP)

    cst8 = np.zeros((P, 2 + 2 * BC), dtype=FP8NP)
    cst8[:, 0:2] = 1.0
    cst8[0, 2:2 + BC] = 1.0  # bias-row rhs: partition 0, j=0 ones
    shared = {
        "W0p": _drpack(W0, FP8NP),
        "W1p": np.ascontiguousarray(
            _drpack(W1, FP8NP).transpose(1, 0, 2, 3)),  # [P, 4, 2, M]
        "W2": (WS * f(inputs["W2"]) * g2).astype(np.float32),
        "W3": (WS * f(inputs["W3"]) * g3).astype(np.float32),
        "Wh0x": np.stack([_drpack(Wh0[g, OUT_B:], FP8NP)
                          for g in range(BLOCKS)]),
        "Wh0d": wh0d,
        "Wh1": wh1,
        "Wgp": np.ascontiguousarray(wgp.transpose(0, 2, 1, 3, 4)),
        "cst": _make_const_block(inputs),
        "cst8": cst8,
    }
    # Wh0x packed as [B, pairs, P, 2, M] -> want [B, P, pairs, 2, M]
    shared["Wh0x"] = np.ascontiguousarray(
        shared["Wh0x"].transpose(0, 2, 1, 3, 4))
    # W0p stays [16, P, 2, M] (indexed by pair in the DMA loop)

    in_maps = []
    for c in range(NCORES):
        sl = slice(c * BC, (c + 1) * BC)
        m = dict(shared)
        dT = np.ascontiguousarray(deter[sl].T)
        m["dT8"] = dT.astype(FP8NP)
        m["dTb"] = dT.astype(_ml.bfloat16)
        m["sT8"] = np.ascontiguousarray(stoch[sl].T).astype(FP8NP)
        m["aT"] = np.ascontiguousarray(action[sl].T)
        m["eT"] = np.ascontiguousarray(d_emb[sl].T)
        in_maps.append(m)
    return in_maps


def _run(inputs, trace=False):
    from concourse import bass_utils
    nc = _get_program()
    in_maps = _prep_inputs(inputs)
    res = bass_utils.run_bass_kernel_spmd(
        nc, in_maps, core_ids=list(range(NCORES)), trace=trace)
    out = np.empty((B, DETER), dtype=np.float32)
    for c in range(NCORES):
        out[c * BC:(c + 1) * BC, :] = \
            np.asarray(res.results[c]["outT"]).astype(np.float32).T
    return out, res.exec_time_ns


def kernel(**inputs):
    out, _ = _run(inputs, trace=False)
    return out


# ---------------------------------------------------------------------------
# benchmarking helper (test-only; the grading path is kernel() above)
# ---------------------------------------------------------------------------

def _bench_generic(nc, in_maps, iters, n_cores=None):
    """Time repeated device executions with device-resident inputs."""
    import time
    import jax
    import concourse.mybir as mybir
    from jax.sharding import Mesh, NamedSharding, PartitionSpec
    from jax.experimental.shard_map import shard_map
    from concourse import bass2jax

    bass2jax.install_neuronx_cc_hook()
    if n_cores is None:
        n_cores = len(in_maps)

    in_names, out_names, out_avals = [], [], []
    for alloc in nc.m.functions[0].allocations:
        if not isinstance(alloc, mybir.MemoryLocationSet):
            continue
        name = alloc.memorylocations[0].name
        pid_name = (nc.partition_id_tensor.name
                    if nc.partition_id_tensor else None)
        if alloc.kind == "ExternalInput":
            if name != pid_name:
                in_names.append(name)
        elif alloc.kind == "ExternalOutput":
            out_names.append(name)
            out_avals.append(jax.core.ShapedArray(
                tuple(alloc.tensor_shape), mybir.dt.np(alloc.dtype)))
    n_params = len(in_names)

    pid_name = nc.partition_id_tensor.name if nc.partition_id_tensor else None
    bind_names = in_names + out_names + ([pid_name] if pid_name else [])

    def _body(*args):
        operands = list(args)
        if pid_name:
            operands.append(bass2jax.partition_id_tensor())
        outs = bass2jax._bass_exec_p.bind(
            *operands,
            out_avals=tuple(out_avals),
            in_names=tuple(bind_names),
            out_names=tuple(out_names),
            lowering_input_output_aliases=(),
            sim_require_finite=True,
            sim_require_nnan=True,
            nc=nc,
        )
        return tuple(outs)

    devices = jax.devices()[:n_cores]
    mesh = Mesh(np.asarray(devices), ("core",))
    nshard = NamedSharding(mesh, PartitionSpec("core"))
    sharded = jax.jit(
        shard_map(_body, mesh=mesh,
                  in_specs=(PartitionSpec("core"),) * (n_params + len(out_names)),
                  out_specs=(PartitionSpec("core"),) * len(out_names),
                  check_rep=False),
        keep_unused=True)

    concat_in = [
        jax.device_put(
            np.concatenate([np.asarray(in_maps[c][nm]) for c in range(n_cores)],
                           axis=0), nshard)
        for nm in in_names]
    concat_zeros = [
        jax.device_put(
            np.zeros((n_cores * a.shape[0], *a.shape[1:]), a.dtype), nshard)
        for a in out_avals]

    outs = sharded(*concat_in, *concat_zeros)
    jax.block_until_ready(outs)

    BATCH = 6
    diffs = []
    for _ in range(iters):
        t0 = time.perf_counter()
        outs = sharded(*concat_in, *concat_zeros)
        jax.block_until_ready(outs)
        t1 = time.perf_counter()
        for _ in range(BATCH):
            outs = sharded(*concat_in, *concat_zeros)
        jax.block_until_ready(outs)
        t2 = time.perf_counter()
        diffs.append((t2 - t1) - (t1 - t0))
    diffs.sort()
    per_iter_ns = diffs[len(diffs) // 2] / (BATCH - 1) * 1e9
    return outs, per_iter_ns


_TINY = None


def _tiny_program():
    """Near-noop program with the SAME input/output signature, to measure
    axon dispatch overhead differentially."""
    global _TINY
    if _TINY is None:
        nc = bacc.Bacc(trn_type="TRN2", target_bir_lowering=False, debug=False)
        d = {"dT8": ((DETER, BC), fp8), "dTb": ((DETER, BC), bf16),
             "sT8": ((STOCH, BC), fp8), "aT": ((ACT_DIM, BC), f32),
             "eT": ((DEMB, BC), f32), "W0p": ((16, P, 2, HIDDEN), fp8),
             "W1p": ((P, 4, 2, HIDDEN), fp8), "W2": ((ACT_DIM, HIDDEN), f32),
             "W3": ((DEMB, HIDDEN), f32),
             "Wh0x": ((BLOCKS, P, 8, 2, OUT_B), fp8),
             "Wh0d": ((BLOCKS, P, 4, OUT_B), bf16),
             "Wh1": ((BLOCKS, P, 4, OUT_B), bf16),
             "Wgp": ((BLOCKS, P, 3, 2, 3 * OUT_B), fp8),
             "cst": ((P, C_NCOL), f32), "cst8": ((P, 2 + 2 * BC), fp8)}
        aps = {k: nc.dram_tensor(k, list(s), dt, kind="ExternalInput").ap()
               for k, (s, dt) in d.items()}
        outT = nc.dram_tensor("outT", [DETER, BC], bf16,
                              kind="ExternalOutput").ap()
        with tile.TileContext(nc) as tc:
            with tc.tile_pool(name="t", bufs=2) as pool:
                t = pool.tile([P, 4, BC], bf16)
                nc.sync.dma_start(
                    out=t, in_=aps["dTb"][:512, :].rearrange(
                        "(s p) b -> p s b", p=P))
                for g in range(BLOCKS):
                    nc.sync.dma_start(
                        out=outT[512 * g:512 * (g + 1), :].rearrange(
                            "(s p) b -> p s b", p=P),
                        in_=t)
        nc.compile()
        _TINY = nc
    return _TINY


def _bench_overhead(inputs, iters=20):
    nc = _tiny_program()
    in_maps = _prep_inputs(inputs)
    _, t = _bench_generic(nc, in_maps, iters)
    return t


def _bench(inputs, iters=20):
    nc = _get_program()
    in_maps = _prep_inputs(inputs)
    outs, per_iter_ns = _bench_generic(nc, in_maps, iters)
    res = np.asarray(outs[0]).reshape(NCORES, DETER, BC)
    out = np.empty((B, DETER), dtype=np.float32)
    for c in range(NCORES):
        out[c * BC:(c + 1) * BC, :] = res[c].astype(np.float32).T
    return out, per_iter_ns
